# revision 7
# baseline (speedup 1.0000x reference)
"""Trainium2 Bass kernel for nn_DAAdj_57114475102829 (GAT-style message passing).

Math (N=4096, F=256, H=8):
  s = x @ Ws.T; t' = x @ Wt.T + b_dist
  z[i,j,h] = s[i,h] + t'[j,h] + (i==j)*selfbias[h]
  out = softmax(relu(z), axis=j) @ W_merge + b_merge

Identity: exp(relu(z)) = max(exp(z),1) = 1 + relu(exp(z)-1), exp(z) = a*e
with a = exp(s) (per-partition scale), e = exp(t') (broadcast row).
Per (row-block, head) ONE fused production instruction:
  Act:  G  = Relu(a*E - 1), accum -> Z-N     (heads 0..nA-1)
  DVE:  EH = max(a*E, 1) via STT, accum -> Z (heads nA..7)
Merge: bf16 PE matmuls psum += diag(wm/Z) @ eh; +1 offset of G-heads folds
into the drain bias K = b_merge + sum_{G-heads} c_h.  Drain: one Act
Identity over all 8 PSUM banks with bias K.  Diagonal selfbias: per-row
corrections (denominator Delta pre-add; output delta applied to the
staged tile after drain, gating only the first 512 output columns' DMA).

Sharding: rows across 8 cores; x rotated per core so the diagonal falls
in columns b*128..(b+1)*128 (bank 0) of each row-block b.
"""
import sys

sys.path.insert(0, "/opt/trn_rl_repo")

import numpy as np
import concourse.bacc as bacc
from concourse import mybir
from concourse.tile import TileContext
from concourse.bass_utils import run_bass_kernel_spmd

N, F, H = 4096, 256, 8
NCORES = 8
ROWS = N // NCORES
P = 128
NB = ROWS // P
JC = 512
NJC = N // JC
FP32 = mybir.dt.float32
BF16 = mybir.dt.bfloat16
AL = mybir.AluOpType
AF = mybir.ActivationFunctionType

NA = 4  # heads produced on Act engine (0..NA-1); DVE gets NA..7

_CACHE = {}


def _build():
    nc = bacc.Bacc("TRN2", target_bir_lowering=False, debug=False, num_devices=NCORES)

    x_d = nc.dram_tensor("x", [N, F], BF16, kind="ExternalInput")
    ws_d = nc.dram_tensor("ws", [2, P, 2 * H], BF16, kind="ExternalInput")
    bd_d = nc.dram_tensor("bd", [H, 1], FP32, kind="ExternalInput")
    bdr_d = nc.dram_tensor("bdr", [1, H], FP32, kind="ExternalInput")
    wm_d = nc.dram_tensor("wm", [1, H], FP32, kind="ExternalInput")
    bm_d = nc.dram_tensor("bm", [1, 1], FP32, kind="ExternalInput")
    sb_d = nc.dram_tensor("sb", [1, H], FP32, kind="ExternalInput")
    out_d = nc.dram_tensor("out", [ROWS, N], FP32, kind="ExternalOutput")

    with TileContext(nc) as tc:
        with tc.tile_pool(name="persist", bufs=1) as persist:
            e_all = persist.tile([P, H, N], BF16, tag="e_all")
            ones = persist.tile([P, N], BF16, tag="ones")
            mask = persist.tile([P, P], FP32, tag="mask")
            maskbf = persist.tile([P, P], BF16, tag="maskbf")
            maskw = persist.tile([P, H, P], BF16, tag="maskw")
            a_all = persist.tile([P, NB, H], FP32, tag="a_all")
            dd8 = persist.tile([P, NB, H], FP32, tag="dd8")
            del8 = persist.tile([P, NB, H], FP32, tag="del8")
            wm_b = persist.tile([P, H], FP32, tag="wm_b")
            bm_c = persist.tile([P, 1], FP32, tag="bm_c")
            sb_b = persist.tile([P, H], FP32, tag="sb_b")
            bdr_b = persist.tile([P, H], FP32, tag="bdr_b")
            bd_c = persist.tile([H, 1], FP32, tag="bd_c")
            neg1 = persist.tile([P, 1], FP32, tag="neg1")
            esb = persist.tile([P, H], FP32, tag="esb")
            it_p = persist.tile([P, 1], FP32, tag="it_p")
            it_f = persist.tile([P, P], FP32, tag="it_f")
            wstt = persist.tile([P, 2, 2 * H], BF16, tag="wstt", name="wstt")

            with tc.tile_pool(name="dram", bufs=1, space="DRAM") as dpool:
                edd = dpool.tile([H, N], BF16)

                with (
                    tc.tile_pool(name="su1", bufs=1) as su1,
                    tc.tile_pool(name="su2", bufs=2) as su2,
                    tc.tile_pool(name="ps_su", bufs=2, space="PSUM") as ps_su,
                ):
                    # ---- x.T first: 2 parallel xbar-transpose DMAs ----
                    xt = [su1.tile([P, N], BF16, tag=f"xt{fh}", name=f"xt{fh}") for fh in range(2)]
                    nc.sync.dma_start_transpose(out=xt[0], in_=x_d[:, 0:P])
                    nc.sync.dma_start_transpose(out=xt[1], in_=x_d[:, P : 2 * P])

                    # ---- params (small DMAs, other ring slots) ----
                    nc.sync.dma_start(out=wstt, in_=ws_d[:, :, :].rearrange("a p c -> p a c"))
                    nc.sync.dma_start(
                        out=wm_b, in_=wm_d[0:1, :].to_broadcast((P, H))
                    )
                    nc.sync.dma_start(
                        out=bm_c, in_=bm_d[0:1, :].to_broadcast((P, 1))
                    )
                    nc.sync.dma_start(
                        out=sb_b, in_=sb_d[0:1, :].to_broadcast((P, H))
                    )
                    nc.sync.dma_start(
                        out=bdr_b, in_=bdr_d[0:1, :].to_broadcast((P, H))
                    )
                    nc.sync.dma_start(out=bd_c, in_=bd_d[:, :])

                    nc.vector.memset(neg1, -1.0)
                    nc.vector.memset(ones, 1.0)
                    nc.gpsimd.iota(
                        it_p, [[0, 1]], channel_multiplier=1,
                        allow_small_or_imprecise_dtypes=True,
                    )
                    nc.gpsimd.iota(
                        it_f, [[1, P]], channel_multiplier=0,
                        allow_small_or_imprecise_dtypes=True,
                    )
                    nc.vector.tensor_scalar(
                        mask, it_f, it_p[:, 0:1], None, AL.is_equal
                    )
                    nc.vector.tensor_copy(maskbf, mask)
                    nc.scalar.activation(esb, sb_b, AF.Exp)
                    for h in range(H):
                        nc.vector.tensor_scalar(
                            maskw[:, h, :], mask, wm_b[:, h : h + 1], None, AL.mult
                        )

                    # ---- t'.T chunks -> exp -> ed; broadcast per chunk ----
                    ed = su1.tile([H, N], BF16, tag="ed")
                    for jc in range(NJC):
                        ps_t = ps_su.tile([H, JC], FP32, tag="ps_t")
                        nc.tensor.matmul(
                            ps_t,
                            lhsT=wstt[:, 0, H : 2 * H],
                            rhs=xt[0][:, jc * JC : (jc + 1) * JC],
                            start=True,
                            stop=False,
                        )
                        nc.tensor.matmul(
                            ps_t,
                            lhsT=wstt[:, 1, H : 2 * H],
                            rhs=xt[1][:, jc * JC : (jc + 1) * JC],
                            start=False,
                            stop=True,
                        )
                        nc.scalar.activation(
                            ed[:, jc * JC : (jc + 1) * JC],
                            ps_t,
                            AF.Exp,
                            bias=bd_c[:, 0:1],
                        )
                        nc.sync.dma_start(
                            out=edd[:, jc * JC : (jc + 1) * JC],
                            in_=ed[:, jc * JC : (jc + 1) * JC],
                        )
                        for h in range(H):
                            nc.sync.dma_start(
                                out=e_all[:, h, jc * JC : (jc + 1) * JC],
                                in_=edd[
                                    h : h + 1, jc * JC : (jc + 1) * JC
                                ].to_broadcast((P, JC)),
                            )

                    # ---- per-block: s, t'_own -> a_all, del8, dd8 ----
                    for b in range(NB):
                        ps8 = ps_su.tile([P, 2 * H], FP32, tag="ps8")
                        nc.tensor.matmul(
                            ps8,
                            lhsT=xt[0][:, b * P : (b + 1) * P],
                            rhs=wstt[:, 0, :],
                            start=True,
                            stop=False,
                        )
                        nc.tensor.matmul(
                            ps8,
                            lhsT=xt[1][:, b * P : (b + 1) * P],
                            rhs=wstt[:, 1, :],
                            start=False,
                            stop=True,
                        )
                        s16 = su2.tile([P, 2 * H], FP32, tag="s16")
                        nc.vector.tensor_copy(s16, ps8)
                        nc.scalar.activation(a_all[:, b, :], s16[:, 0:H], AF.Exp)
                        ts = su2.tile([P, H], FP32, tag="ts")
                        nc.vector.tensor_tensor(
                            out=ts, in0=s16[:, 0:H], in1=s16[:, H : 2 * H], op=AL.add
                        )
                        v = su2.tile([P, H], FP32, tag="v")
                        nc.vector.tensor_tensor(out=v, in0=ts, in1=bdr_b, op=AL.add)
                        u = su2.tile([P, H], FP32, tag="u")
                        nc.scalar.activation(u, v, AF.Exp)
                        ue = su2.tile([P, H], FP32, tag="ue")
                        nc.vector.tensor_tensor(out=ue, in0=u, in1=esb, op=AL.mult)
                        m1 = su2.tile([P, H], FP32, tag="m1")
                        nc.vector.tensor_scalar(m1, u, 1.0, None, AL.max)
                        m2 = su2.tile([P, H], FP32, tag="m2")
                        nc.vector.tensor_scalar(m2, ue, 1.0, None, AL.max)
                        nc.vector.tensor_tensor(
                            out=del8[:, b, :], in0=m2, in1=m1, op=AL.subtract
                        )
                        nc.vector.tensor_scalar(
                            dd8[:, b, 0:NA], del8[:, b, 0:NA], float(N), None, AL.add
                        )
                        nc.vector.tensor_copy(dd8[:, b, NA:H], del8[:, b, NA:H])

                # ================= steady state =================
                with (
                    tc.tile_pool(name="big", bufs=6) as big,
                    tc.tile_pool(name="dcp", bufs=4) as dcp,
                    tc.tile_pool(name="small", bufs=2) as small,
                    tc.tile_pool(name="ost", bufs=2) as ost,
                    tc.tile_pool(name="mps", bufs=1, space="PSUM") as mps,
                ):
                    ND = H - NA
                    for b in range(NB):
                        psum = mps.tile([P, N], FP32, tag="psum", name=f"psum{b}")
                        acc8 = small.tile([P, H], FP32, tag="acc8")
                        z8 = small.tile([P, H], FP32, tag="z8")
                        r8 = small.tile([P, H], FP32, tag="r8")
                        ehs = {}
                        dcs = {}

                        def produce(h):
                            eh = big.tile([P, N], BF16, tag="eh")
                            ehs[h] = eh
                            if h < NA:
                                nc.scalar.activation(
                                    eh,
                                    e_all[:, h, :],
                                    AF.Relu,
                                    bias=neg1[:, 0:1],
                                    scale=a_all[:, b, h : h + 1],
                                    accum_out=acc8[:, h : h + 1],
                                )
                            else:
                                nc.vector.scalar_tensor_tensor(
                                    eh,
                                    e_all[:, h, :],
                                    a_all[:, b, h : h + 1],
                                    ones,
                                    AL.mult,
                                    AL.max,
                                    accum_out=acc8[:, h : h + 1],
                                )

                        def zr(h):
                            nc.vector.tensor_tensor(
                                out=z8[:, h : h + 1],
                                in0=acc8[:, h : h + 1],
                                in1=dd8[:, b, h : h + 1],
                                op=AL.add,
                            )
                            nc.vector.reciprocal(
                                r8[:, h : h + 1], z8[:, h : h + 1]
                            )

                        def dc_act(h):
                            dc = dcp.tile([P, P], BF16, tag="dc")
                            dcs[h] = dc
                            nc.scalar.activation(
                                dc,
                                maskw[:, h, :],
                                AF.Identity,
                                scale=r8[:, h : h + 1],
                            )

                        def dc_dve(h):
                            dc = dcp.tile([P, P], BF16, tag="dc")
                            dcs[h] = dc
                            nc.vector.tensor_scalar(
                                dc, maskw[:, h, :], r8[:, h : h + 1], None, AL.mult
                            )

                        def merge(h, first, last):
                            for jc in range(NJC):
                                nc.tensor.matmul(
                                    psum[:, jc * JC : (jc + 1) * JC],
                                    lhsT=dcs[h],
                                    rhs=ehs[h][:, jc * JC : (jc + 1) * JC],
                                    start=first,
                                    stop=last,
                                )

                        c8a = small.tile([P, NA], FP32, tag="c8a")
                        k1 = small.tile([P, 1], FP32, tag="k1")
                        kb = small.tile([P, 1], FP32, tag="kb")

                        # ---- emission order (per-engine FIFOs matter) ----
                        # Act: P_A0 P_A1 dcA0 P_A2 dcA1 P_A3 dcA2 dcA3 drain
                        # DVE: S_D0 zrA0 S_D1 zrD0 dcD0 zrA1 S_D2 zrD1 dcD1
                        #      zrA2 S_D3 zrD2 dcD2 zrA3 c8a k1 kb zrD3 dcD3 ...
                        produce(0)          # Act
                        produce(NA)         # DVE
                        produce(1)          # Act
                        zr(0)
                        produce(NA + 1)     # DVE
                        zr(NA)
                        dc_dve(NA)
                        dc_act(0)
                        merge(0, True, False)
                        produce(2)          # Act
                        zr(1)
                        dc_act(1)
                        merge(NA, False, False)
                        merge(1, False, False)
                        produce(NA + 2)     # DVE
                        zr(NA + 1)
                        dc_dve(NA + 1)
                        merge(NA + 1, False, False)
                        produce(3)          # Act
                        zr(2)
                        dc_act(2)
                        merge(2, False, False)
                        produce(NA + 3)     # DVE
                        zr(NA + 2)
                        dc_dve(NA + 2)
                        merge(NA + 2, False, False)
                        zr(3)
                        dc_act(3)
                        merge(3, False, False)
                        # K bias (needs act-head recips only)
                        nc.vector.tensor_tensor(
                            out=c8a, in0=r8[:, 0:NA], in1=wm_b[:, 0:NA], op=AL.mult
                        )
                        nc.vector.tensor_reduce(
                            k1, c8a, axis=mybir.AxisListType.X, op=AL.add
                        )
                        nc.vector.tensor_tensor(out=kb, in0=k1, in1=bm_c, op=AL.add)
                        zr(NA + 3)
                        dc_dve(NA + 3)
                        merge(NA + 3, False, True)

                        # drain all 8 banks in one Act op (+K bias)
                        stage = ost.tile([P, N], FP32, tag="stage")
                        nc.scalar.activation(
                            stage, psum, AF.Identity, bias=kb[:, 0:1]
                        )
                        # diag fix on stage cols [b*128, b*128+128)
                        c8 = small.tile([P, H], FP32, tag="c8")
                        nc.vector.tensor_tensor(out=c8, in0=r8, in1=wm_b, op=AL.mult)
                        t8 = small.tile([P, H], FP32, tag="t8")
                        nc.vector.tensor_tensor(
                            out=t8, in0=c8, in1=del8[:, b, :], op=AL.mult
                        )
                        dlt = small.tile([P, 1], FP32, tag="dlt")
                        nc.vector.tensor_reduce(
                            dlt, t8, axis=mybir.AxisListType.X, op=AL.add
                        )
                        nc.vector.scalar_tensor_tensor(
                            stage[:, b * P : (b + 1) * P],
                            maskbf,
                            dlt[:, 0:1],
                            stage[:, b * P : (b + 1) * P],
                            AL.mult,
                            AL.add,
                        )
                        # cols 512.. go out right after drain; cols 0..512
                        # wait for the diag fix
                        nc.sync.dma_start(
                            out=out_d[b * P : (b + 1) * P, JC:N],
                            in_=stage[:, JC:N],
                        )
                        nc.sync.dma_start(
                            out=out_d[b * P : (b + 1) * P, 0:JC],
                            in_=stage[:, 0:JC],
                        )

    nc.compile()
    return nc


def _get_nc():
    if "nc" not in _CACHE:
        _CACHE["nc"] = _build()
    return _CACHE["nc"]


def _in_maps(inputs):
    import ml_dtypes

    x = np.ascontiguousarray(np.asarray(inputs["x"], dtype=np.float32))
    W_dist = np.asarray(inputs["W_dist"], dtype=np.float32)
    b_dist = np.asarray(inputs["b_dist"], dtype=np.float32).reshape(H, 1)
    W_merge = np.asarray(inputs["W_merge"], dtype=np.float32).reshape(1, H)
    b_merge = np.asarray(inputs["b_merge"], dtype=np.float32).reshape(1, 1)
    selfbias = np.asarray(inputs["selfbias"], dtype=np.float32).reshape(1, H)
    # wstt[fh] = [Ws.T | Wt.T] block fh: [128, 2H]
    Ws = W_dist[:, :F]
    Wt = W_dist[:, F:]
    wstt = np.empty((2, P, 2 * H), dtype=np.float32)
    for fh in range(2):
        wstt[fh, :, 0:H] = Ws[:, fh * P : (fh + 1) * P].T
        wstt[fh, :, H : 2 * H] = Wt[:, fh * P : (fh + 1) * P].T
    wstt = wstt.astype(ml_dtypes.bfloat16)
    in_maps = []
    for c in range(NCORES):
        xr = np.roll(x, -c * ROWS, axis=0).astype(ml_dtypes.bfloat16)
        in_maps.append(
            {
                "x": np.ascontiguousarray(xr),
                "ws": wstt,
                "bd": b_dist,
                "bdr": np.ascontiguousarray(b_dist.reshape(1, H)),
                "wm": W_merge,
                "bm": b_merge,
                "sb": selfbias,
            }
        )
    return in_maps


def _assemble(results):
    out = np.empty((N, N), dtype=np.float32)
    for c in range(NCORES):
        out[c * ROWS : (c + 1) * ROWS, :] = np.roll(
            results[c]["out"], c * ROWS, axis=1
        )
    return out


def kernel(x, W_dist, b_dist, W_merge, b_merge, selfbias):
    nc = _get_nc()
    in_maps = _in_maps(
        {
            "x": x,
            "W_dist": W_dist,
            "b_dist": b_dist,
            "W_merge": W_merge,
            "b_merge": b_merge,
            "selfbias": selfbias,
        }
    )
    res = run_bass_kernel_spmd(nc, in_maps, core_ids=list(range(NCORES)))
    return _assemble(res.results)


# revision 9
# speedup vs baseline: 1.5118x; 1.5118x over previous
"""Trainium2 Bass kernel for nn_DAAdj_57114475102829 (GAT-style message passing).

Math (N=4096, F=256, H=8):
  s = x @ Ws.T; t' = x @ Wt.T + b_dist
  z[i,j,h] = s[i,h] + t'[j,h] + (i==j)*selfbias[h]
  out = softmax(relu(z), axis=j) @ W_merge + b_merge

Identity: exp(relu(z)) = max(exp(z),1) = 1 + relu(exp(z)-1), exp(z) = a*e
with a = exp(s) (per-partition scale), e = exp(t') (broadcast row).
Per (row-block, head) ONE fused production instruction:
  Act:  G  = Relu(a*E - 1), accum -> Z-N     (heads 0..nA-1)
  DVE:  EH = max(a*E, 1) via STT, accum -> Z (heads nA..7)
Merge: bf16 PE matmuls psum += diag(wm/Z) @ eh; +1 offset of G-heads folds
into the drain bias K = b_merge + sum_{G-heads} c_h.  Drain: one Act
Identity over all 8 PSUM banks with bias K.  Diagonal selfbias: per-row
corrections (denominator Delta pre-add; output delta applied to the
staged tile after drain, gating only the first 512 output columns' DMA).

Sharding: rows across 8 cores; x rotated per core so the diagonal falls
in columns b*128..(b+1)*128 (bank 0) of each row-block b.
"""
import sys

sys.path.insert(0, "/opt/trn_rl_repo")

import numpy as np
import concourse.bacc as bacc
from concourse import mybir
from concourse.tile import TileContext
from concourse.bass_utils import run_bass_kernel_spmd

N, F, H = 4096, 256, 8
NCORES = 8
ROWS = N // NCORES
P = 128
NB = ROWS // P
JC = 512
NJC = N // JC
FP32 = mybir.dt.float32
BF16 = mybir.dt.bfloat16
AL = mybir.AluOpType
AF = mybir.ActivationFunctionType

NA = 4  # heads produced on Act engine (0..NA-1); DVE gets NA..7

_CACHE = {}


def _build():
    nc = bacc.Bacc("TRN2", target_bir_lowering=False, debug=False, num_devices=NCORES)

    x_d = nc.dram_tensor("x", [N, F], BF16, kind="ExternalInput")
    ws_d = nc.dram_tensor("ws", [2, P, 2 * H], BF16, kind="ExternalInput")
    bd_d = nc.dram_tensor("bd", [H, 1], FP32, kind="ExternalInput")
    bdr_d = nc.dram_tensor("bdr", [1, H], FP32, kind="ExternalInput")
    wm_d = nc.dram_tensor("wm", [1, H], FP32, kind="ExternalInput")
    bm_d = nc.dram_tensor("bm", [1, 1], FP32, kind="ExternalInput")
    sb_d = nc.dram_tensor("sb", [1, H], FP32, kind="ExternalInput")
    out_d = nc.dram_tensor("out", [ROWS, N], FP32, kind="ExternalOutput")

    with TileContext(nc) as tc:
        with tc.tile_pool(name="persist", bufs=1) as persist:
            e_all = persist.tile([P, H, N], BF16, tag="e_all")
            ones = persist.tile([P, N], BF16, tag="ones")
            mask = persist.tile([P, P], FP32, tag="mask")
            maskbf = persist.tile([P, P], BF16, tag="maskbf")
            maskw = persist.tile([P, H, P], BF16, tag="maskw")
            a_all = persist.tile([P, NB, H], FP32, tag="a_all")
            dd8 = persist.tile([P, NB, H], FP32, tag="dd8")
            del8 = persist.tile([P, NB, H], FP32, tag="del8")
            wm_b = persist.tile([P, H], FP32, tag="wm_b")
            bm_c = persist.tile([P, 1], FP32, tag="bm_c")
            sb_b = persist.tile([P, H], FP32, tag="sb_b")
            bdr_b = persist.tile([P, H], FP32, tag="bdr_b")
            bd_c = persist.tile([H, 1], FP32, tag="bd_c")
            neg1 = persist.tile([P, 1], FP32, tag="neg1")
            esb = persist.tile([P, H], FP32, tag="esb")
            it_p = persist.tile([P, 1], FP32, tag="it_p")
            it_f = persist.tile([P, P], FP32, tag="it_f")
            wstt = persist.tile([P, 2, 2 * H], BF16, tag="wstt", name="wstt")

            with tc.tile_pool(name="dram", bufs=1, space="DRAM") as dpool:
                edd = dpool.tile([H, N], BF16)

                with (
                    tc.tile_pool(name="su1", bufs=1) as su1,
                    tc.tile_pool(name="su2", bufs=2) as su2,
                    tc.tile_pool(name="ps_su", bufs=2, space="PSUM") as ps_su,
                ):
                    # ---- x.T first: 2 parallel xbar-transpose DMAs ----
                    xt = [su1.tile([P, N], BF16, tag=f"xt{fh}", name=f"xt{fh}") for fh in range(2)]
                    nc.sync.dma_start_transpose(out=xt[0], in_=x_d[:, 0:P])
                    nc.sync.dma_start_transpose(out=xt[1], in_=x_d[:, P : 2 * P])

                    # ---- params (small DMAs, other ring slots) ----
                    nc.sync.dma_start(out=wstt, in_=ws_d[:, :, :].rearrange("a p c -> p a c"))
                    nc.sync.dma_start(
                        out=wm_b, in_=wm_d[0:1, :].to_broadcast((P, H))
                    )
                    nc.sync.dma_start(
                        out=bm_c, in_=bm_d[0:1, :].to_broadcast((P, 1))
                    )
                    nc.sync.dma_start(
                        out=sb_b, in_=sb_d[0:1, :].to_broadcast((P, H))
                    )
                    nc.sync.dma_start(
                        out=bdr_b, in_=bdr_d[0:1, :].to_broadcast((P, H))
                    )
                    nc.sync.dma_start(out=bd_c, in_=bd_d[:, :])

                    nc.vector.memset(neg1, -1.0)
                    nc.vector.memset(ones, 1.0)
                    nc.gpsimd.iota(
                        it_p, [[0, 1]], channel_multiplier=1,
                        allow_small_or_imprecise_dtypes=True,
                    )
                    nc.gpsimd.iota(
                        it_f, [[1, P]], channel_multiplier=0,
                        allow_small_or_imprecise_dtypes=True,
                    )
                    nc.vector.tensor_scalar(
                        mask, it_f, it_p[:, 0:1], None, AL.is_equal
                    )
                    nc.vector.tensor_copy(maskbf, mask)
                    nc.scalar.activation(esb, sb_b, AF.Exp)
                    for h in range(H):
                        nc.vector.tensor_scalar(
                            maskw[:, h, :], mask, wm_b[:, h : h + 1], None, AL.mult
                        )

                    # ---- t'.T chunks -> exp -> ed; broadcast per chunk ----
                    ed = su1.tile([H, N], BF16, tag="ed")
                    for jc in range(NJC):
                        ps_t = ps_su.tile([H, JC], FP32, tag="ps_t")
                        nc.tensor.matmul(
                            ps_t,
                            lhsT=wstt[:, 0, H : 2 * H],
                            rhs=xt[0][:, jc * JC : (jc + 1) * JC],
                            start=True,
                            stop=False,
                        )
                        nc.tensor.matmul(
                            ps_t,
                            lhsT=wstt[:, 1, H : 2 * H],
                            rhs=xt[1][:, jc * JC : (jc + 1) * JC],
                            start=False,
                            stop=True,
                        )
                        nc.scalar.activation(
                            ed[:, jc * JC : (jc + 1) * JC],
                            ps_t,
                            AF.Exp,
                            bias=bd_c[:, 0:1],
                        )

                    nc.sync.dma_start(out=edd, in_=ed)
                    for h in range(H):
                        eng = nc.sync if h % 2 == 0 else nc.scalar
                        eng.dma_start(
                            out=e_all[:, h, :],
                            in_=edd[h : h + 1, :].to_broadcast((P, N)),
                        )
                    # ---- per-block: s, t'_own -> a_all, del8, dd8 ----
                    for b in range(NB):
                        ps8 = ps_su.tile([P, 2 * H], FP32, tag="ps8")
                        nc.tensor.matmul(
                            ps8,
                            lhsT=xt[0][:, b * P : (b + 1) * P],
                            rhs=wstt[:, 0, :],
                            start=True,
                            stop=False,
                        )
                        nc.tensor.matmul(
                            ps8,
                            lhsT=xt[1][:, b * P : (b + 1) * P],
                            rhs=wstt[:, 1, :],
                            start=False,
                            stop=True,
                        )
                        s16 = su2.tile([P, 2 * H], FP32, tag="s16")
                        nc.vector.tensor_copy(s16, ps8)
                        nc.scalar.activation(a_all[:, b, :], s16[:, 0:H], AF.Exp)
                        ts = su2.tile([P, H], FP32, tag="ts")
                        nc.vector.tensor_tensor(
                            out=ts, in0=s16[:, 0:H], in1=s16[:, H : 2 * H], op=AL.add
                        )
                        v = su2.tile([P, H], FP32, tag="v")
                        nc.vector.tensor_tensor(out=v, in0=ts, in1=bdr_b, op=AL.add)
                        u = su2.tile([P, H], FP32, tag="u")
                        nc.scalar.activation(u, v, AF.Exp)
                        ue = su2.tile([P, H], FP32, tag="ue")
                        nc.vector.tensor_tensor(out=ue, in0=u, in1=esb, op=AL.mult)
                        m1 = su2.tile([P, H], FP32, tag="m1")
                        nc.vector.tensor_scalar(m1, u, 1.0, None, AL.max)
                        m2 = su2.tile([P, H], FP32, tag="m2")
                        nc.vector.tensor_scalar(m2, ue, 1.0, None, AL.max)
                        nc.vector.tensor_tensor(
                            out=del8[:, b, :], in0=m2, in1=m1, op=AL.subtract
                        )
                        nc.vector.tensor_scalar(
                            dd8[:, b, 0:NA], del8[:, b, 0:NA], float(N), None, AL.add
                        )
                        nc.vector.tensor_copy(dd8[:, b, NA:H], del8[:, b, NA:H])

                # ================= steady state =================
                with (
                    tc.tile_pool(name="big", bufs=6) as big,
                    tc.tile_pool(name="dcp", bufs=4) as dcp,
                    tc.tile_pool(name="small", bufs=2) as small,
                    tc.tile_pool(name="ost", bufs=2) as ost,
                    tc.tile_pool(name="mps", bufs=1, space="PSUM") as mps,
                ):
                    ND = H - NA

                    def make_block(b):
                        st = {}
                        st["b"] = b
                        st["psum"] = mps.tile(
                            [P, N], FP32, tag="psum", name=f"psum{b}"
                        )
                        st["acc8"] = small.tile(
                            [P, H], FP32, tag="acc8", name=f"acc8_{b}"
                        )
                        st["z8"] = small.tile([P, H], FP32, tag="z8", name=f"z8_{b}")
                        st["r8"] = small.tile([P, H], FP32, tag="r8", name=f"r8_{b}")
                        st["kb"] = small.tile([P, 1], FP32, tag="kb", name=f"kb_{b}")
                        st["stage"] = ost.tile(
                            [P, N], FP32, tag="stage", name=f"stage_{b}"
                        )
                        st["ehs"] = {}
                        st["dcs"] = {}
                        return st

                    def produce(st, h):
                        eh = big.tile([P, N], BF16, tag="eh", name=f"eh{st['b']}_{h}")
                        st["ehs"][h] = eh
                        if h < NA:
                            nc.scalar.activation(
                                eh,
                                e_all[:, h, :],
                                AF.Relu,
                                bias=neg1[:, 0:1],
                                scale=a_all[:, st["b"], h : h + 1],
                                accum_out=st["acc8"][:, h : h + 1],
                            )
                        else:
                            nc.vector.scalar_tensor_tensor(
                                eh,
                                e_all[:, h, :],
                                a_all[:, st["b"], h : h + 1],
                                ones,
                                AL.mult,
                                AL.max,
                                accum_out=st["acc8"][:, h : h + 1],
                            )

                    def zr(st, h):
                        nc.vector.tensor_tensor(
                            out=st["z8"][:, h : h + 1],
                            in0=st["acc8"][:, h : h + 1],
                            in1=dd8[:, st["b"], h : h + 1],
                            op=AL.add,
                        )
                        nc.vector.reciprocal(
                            st["r8"][:, h : h + 1], st["z8"][:, h : h + 1]
                        )

                    def dc_act(st, h):
                        dc = dcp.tile([P, P], BF16, tag="dc", name=f"dc{st['b']}_{h}")
                        st["dcs"][h] = dc
                        nc.scalar.activation(
                            dc, maskw[:, h, :], AF.Identity,
                            scale=st["r8"][:, h : h + 1],
                        )

                    def dc_dve(st, h):
                        dc = dcp.tile([P, P], BF16, tag="dc", name=f"dc{st['b']}_{h}")
                        st["dcs"][h] = dc
                        nc.vector.tensor_scalar(
                            dc, maskw[:, h, :], st["r8"][:, h : h + 1], None, AL.mult
                        )

                    BANKS = [4, 5, 6, 7, 0, 1, 2, 3]

                    def merge(st, h, first, last):
                        for jc in BANKS:
                            nc.tensor.matmul(
                                st["psum"][:, jc * JC : (jc + 1) * JC],
                                lhsT=st["dcs"][h],
                                rhs=st["ehs"][h][:, jc * JC : (jc + 1) * JC],
                                start=first,
                                stop=last,
                            )

                    def kbias(st):
                        b = st["b"]
                        c8a = small.tile([P, NA], FP32, tag="c8a", name=f"c8a_{b}")
                        k1 = small.tile([P, 1], FP32, tag="k1", name=f"k1_{b}")
                        nc.vector.tensor_tensor(
                            out=c8a, in0=st["r8"][:, 0:NA], in1=wm_b[:, 0:NA],
                            op=AL.mult,
                        )
                        nc.vector.tensor_reduce(
                            k1, c8a, axis=mybir.AxisListType.X, op=AL.add
                        )
                        nc.vector.tensor_tensor(
                            out=st["kb"], in0=k1, in1=bm_c, op=AL.add
                        )

                    def drain_hi(st):
                        # banks 4..7 = cols 2048..4096
                        nc.scalar.activation(
                            st["stage"][:, N // 2 : N],
                            st["psum"][:, N // 2 : N],
                            AF.Identity,
                            bias=st["kb"][:, 0:1],
                        )

                    def drain_lo(st):
                        nc.scalar.activation(
                            st["stage"][:, 0 : N // 2],
                            st["psum"][:, 0 : N // 2],
                            AF.Identity,
                            bias=st["kb"][:, 0:1],
                        )

                    def tail_dve(st):
                        b = st["b"]
                        c8 = small.tile([P, H], FP32, tag="c8", name=f"c8_{b}")
                        t8 = small.tile([P, H], FP32, tag="t8", name=f"t8_{b}")
                        dlt = small.tile([P, 1], FP32, tag="dlt", name=f"dlt_{b}")
                        nc.vector.tensor_tensor(
                            out=c8, in0=st["r8"], in1=wm_b, op=AL.mult
                        )
                        nc.vector.tensor_tensor(
                            out=t8, in0=c8, in1=del8[:, b, :], op=AL.mult
                        )
                        nc.vector.tensor_reduce(
                            dlt, t8, axis=mybir.AxisListType.X, op=AL.add
                        )
                        nc.vector.scalar_tensor_tensor(
                            st["stage"][:, b * P : (b + 1) * P],
                            maskbf,
                            dlt[:, 0:1],
                            st["stage"][:, b * P : (b + 1) * P],
                            AL.mult,
                            AL.add,
                        )

                    def tail_dma(st):
                        b = st["b"]
                        nc.sync.dma_start(
                            out=out_d[b * P : (b + 1) * P, N // 2 : N],
                            in_=st["stage"][:, N // 2 : N],
                        )
                        nc.sync.dma_start(
                            out=out_d[b * P : (b + 1) * P, 0 : N // 2],
                            in_=st["stage"][:, 0 : N // 2],
                        )

                    prev = None
                    for b in range(NB):
                        st = make_block(b)
                        produce(st, 0)       # Act
                        produce(st, NA)      # DVE
                        produce(st, 1)       # Act
                        zr(st, 0)
                        if prev is not None:
                            drain_hi(prev)   # Act queue slot
                            tail_dve(prev)   # needs drain_lo(prev) (done earlier)
                            tail_dma(prev)
                        produce(st, NA + 1)  # DVE
                        zr(st, NA)
                        dc_dve(st, NA)
                        dc_act(st, 0)
                        merge(st, 0, True, False)
                        produce(st, 2)       # Act
                        zr(st, 1)
                        dc_act(st, 1)
                        merge(st, NA, False, False)
                        merge(st, 1, False, False)
                        produce(st, NA + 2)  # DVE
                        zr(st, NA + 1)
                        dc_dve(st, NA + 1)
                        merge(st, NA + 1, False, False)
                        produce(st, 3)       # Act
                        zr(st, 2)
                        dc_act(st, 2)
                        merge(st, 2, False, False)
                        produce(st, NA + 3)  # DVE
                        zr(st, NA + 2)
                        dc_dve(st, NA + 2)
                        merge(st, NA + 2, False, False)
                        zr(st, 3)
                        dc_act(st, 3)
                        merge(st, 3, False, False)
                        kbias(st)
                        zr(st, NA + 3)
                        dc_dve(st, NA + 3)
                        merge(st, NA + 3, False, True)
                        drain_lo(st)         # bank 0..3 right after last merge
                        prev = st
                    drain_hi(prev)
                    tail_dve(prev)
                    tail_dma(prev)

    nc.compile()
    return nc


def _get_nc():
    if "nc" not in _CACHE:
        _CACHE["nc"] = _build()
    return _CACHE["nc"]


def _in_maps(inputs):
    import ml_dtypes

    x = np.ascontiguousarray(np.asarray(inputs["x"], dtype=np.float32))
    W_dist = np.asarray(inputs["W_dist"], dtype=np.float32)
    b_dist = np.asarray(inputs["b_dist"], dtype=np.float32).reshape(H, 1)
    W_merge = np.asarray(inputs["W_merge"], dtype=np.float32).reshape(1, H)
    b_merge = np.asarray(inputs["b_merge"], dtype=np.float32).reshape(1, 1)
    selfbias = np.asarray(inputs["selfbias"], dtype=np.float32).reshape(1, H)
    # wstt[fh] = [Ws.T | Wt.T] block fh: [128, 2H]
    Ws = W_dist[:, :F]
    Wt = W_dist[:, F:]
    wstt = np.empty((2, P, 2 * H), dtype=np.float32)
    for fh in range(2):
        wstt[fh, :, 0:H] = Ws[:, fh * P : (fh + 1) * P].T
        wstt[fh, :, H : 2 * H] = Wt[:, fh * P : (fh + 1) * P].T
    wstt = wstt.astype(ml_dtypes.bfloat16)
    in_maps = []
    for c in range(NCORES):
        xr = np.roll(x, -c * ROWS, axis=0).astype(ml_dtypes.bfloat16)
        in_maps.append(
            {
                "x": np.ascontiguousarray(xr),
                "ws": wstt,
                "bd": b_dist,
                "bdr": np.ascontiguousarray(b_dist.reshape(1, H)),
                "wm": W_merge,
                "bm": b_merge,
                "sb": selfbias,
            }
        )
    return in_maps


def _assemble(results):
    out = np.empty((N, N), dtype=np.float32)
    for c in range(NCORES):
        out[c * ROWS : (c + 1) * ROWS, :] = np.roll(
            results[c]["out"], c * ROWS, axis=1
        )
    return out


def kernel(x, W_dist, b_dist, W_merge, b_merge, selfbias):
    nc = _get_nc()
    in_maps = _in_maps(
        {
            "x": x,
            "W_dist": W_dist,
            "b_dist": b_dist,
            "W_merge": W_merge,
            "b_merge": b_merge,
            "selfbias": selfbias,
        }
    )
    res = run_bass_kernel_spmd(nc, in_maps, core_ids=list(range(NCORES)))
    return _assemble(res.results)


# revision 12
# speedup vs baseline: 1.5250x; 1.0087x over previous
"""Trainium2 Bass kernel for nn_DAAdj_57114475102829 (GAT-style message passing).

Math (N=4096, F=256, H=8):
  s = x @ Ws.T; t' = x @ Wt.T + b_dist
  z[i,j,h] = s[i,h] + t'[j,h] + (i==j)*selfbias[h]
  out = softmax(relu(z), axis=j) @ W_merge + b_merge

Identity: exp(relu(z)) = max(exp(z),1) = 1 + relu(exp(z)-1), exp(z) = a*e
with a = exp(s) (per-partition scale), e = exp(t') (broadcast row).
Per (row-block, head) ONE fused production instruction:
  Act:  G  = Relu(a*E - 1), accum -> Z-N     (heads 0..NA-1)
  DVE:  EH = max(a*E, 1) via STT, accum -> Z (heads NA..7)
Merge: bf16 PE matmuls psum += diag(wm/Z) @ eh; +1 offset of G-heads folds
into the drain bias K = b_merge + sum_{G-heads} c_h.  Drains are two Act
Identity ops (banks 0-3 / 4-7) software-pipelined into the next block's
instruction stream.  Diagonal selfbias enters as per-row corrections
(denominator Delta pre-add; output delta applied to the staged tile).

Sharding: rows across 8 cores; x rotated per core so the diagonal falls
in columns b*128..(b+1)*128 (bank 0) of each row-block b.
"""
import sys

sys.path.insert(0, "/opt/trn_rl_repo")

import numpy as np
import concourse.bacc as bacc
from concourse import mybir
from concourse.tile import TileContext
from concourse.bass_utils import run_bass_kernel_spmd

N, F, H = 4096, 256, 8
NCORES = 8
ROWS = N // NCORES
P = 128
NB = ROWS // P
JC = 512
NJC = N // JC
FP32 = mybir.dt.float32
BF16 = mybir.dt.bfloat16
AL = mybir.AluOpType
AF = mybir.ActivationFunctionType

NA = 4  # heads produced on Act engine (0..NA-1); DVE gets NA..7

_CACHE = {}


def _build():
    nc = bacc.Bacc("TRN2", target_bir_lowering=False, debug=False, num_devices=NCORES)

    x_d = nc.dram_tensor("x", [N, F], BF16, kind="ExternalInput")
    ws_d = nc.dram_tensor("ws", [2, P, 2 * H], BF16, kind="ExternalInput")
    bd_d = nc.dram_tensor("bd", [H, 1], FP32, kind="ExternalInput")
    bdr_d = nc.dram_tensor("bdr", [1, H], FP32, kind="ExternalInput")
    wm_d = nc.dram_tensor("wm", [1, H], FP32, kind="ExternalInput")
    bm_d = nc.dram_tensor("bm", [1, 1], FP32, kind="ExternalInput")
    sb_d = nc.dram_tensor("sb", [1, H], FP32, kind="ExternalInput")
    out_d = nc.dram_tensor("out", [ROWS, N], FP32, kind="ExternalOutput")

    with TileContext(nc) as tc:
        with tc.tile_pool(name="persist", bufs=1) as persist:
            e_all = persist.tile([P, H, N], BF16, tag="e_all")
            ones = persist.tile([P, N], BF16, tag="ones")
            mask = persist.tile([P, P], FP32, tag="mask")
            maskbf = persist.tile([P, P], BF16, tag="maskbf")
            maskw = persist.tile([P, H, P], BF16, tag="maskw")
            a_all = persist.tile([P, NB, H], FP32, tag="a_all")
            dd8 = persist.tile([P, NB, H], FP32, tag="dd8")
            del8 = persist.tile([P, NB, H], FP32, tag="del8")
            wm_b = persist.tile([P, H], FP32, tag="wm_b")
            bm_c = persist.tile([P, 1], FP32, tag="bm_c")
            sb_b = persist.tile([P, H], FP32, tag="sb_b")
            bdr_b = persist.tile([P, H], FP32, tag="bdr_b")
            bd_c = persist.tile([H, 1], FP32, tag="bd_c")
            neg1 = persist.tile([P, 1], FP32, tag="neg1")
            esb = persist.tile([P, H], FP32, tag="esb")
            it_p = persist.tile([P, 1], FP32, tag="it_p")
            it_f = persist.tile([P, P], FP32, tag="it_f")
            wstt = persist.tile([P, 2, 2 * H], BF16, tag="wstt", name="wstt")

            with tc.tile_pool(name="dram", bufs=1, space="DRAM") as dpool:
                edd = dpool.tile([H, N], BF16)

                with (
                    tc.tile_pool(name="su1", bufs=1) as su1,
                    tc.tile_pool(name="su2", bufs=2) as su2,
                    tc.tile_pool(name="ps_su", bufs=1, space="PSUM") as ps_su,
                    tc.tile_pool(name="ps_s8", bufs=2, space="PSUM") as ps_s8,
                ):
                    # ---- x.T first: xbar-transpose DMAs ----
                    xt = [
                        su1.tile([P, N], BF16, tag=f"xt{fh}", name=f"xt{fh}")
                        for fh in range(2)
                    ]
                    nc.sync.dma_start_transpose(out=xt[0], in_=x_d[:, 0:P])
                    nc.sync.dma_start_transpose(out=xt[1], in_=x_d[:, P : 2 * P])

                    # ---- params (scalar-engine ring; sync ring busy) ----
                    nc.scalar.dma_start(
                        out=wstt, in_=ws_d[:, :, :].rearrange("a p c -> p a c")
                    )
                    nc.scalar.dma_start(
                        out=wm_b, in_=wm_d[0:1, :].to_broadcast((P, H))
                    )
                    nc.scalar.dma_start(
                        out=bm_c, in_=bm_d[0:1, :].to_broadcast((P, 1))
                    )
                    nc.scalar.dma_start(
                        out=sb_b, in_=sb_d[0:1, :].to_broadcast((P, H))
                    )
                    nc.scalar.dma_start(
                        out=bdr_b, in_=bdr_d[0:1, :].to_broadcast((P, H))
                    )
                    nc.scalar.dma_start(out=bd_c, in_=bd_d[:, :])

                    nc.vector.memset(neg1, -1.0)
                    nc.vector.memset(ones, 1.0)
                    nc.gpsimd.iota(
                        it_p, [[0, 1]], channel_multiplier=1,
                        allow_small_or_imprecise_dtypes=True,
                    )
                    nc.gpsimd.iota(
                        it_f, [[1, P]], channel_multiplier=0,
                        allow_small_or_imprecise_dtypes=True,
                    )
                    nc.vector.tensor_scalar(
                        mask, it_f, it_p[:, 0:1], None, AL.is_equal
                    )
                    nc.vector.tensor_copy(maskbf, mask)
                    nc.scalar.activation(esb, sb_b, AF.Exp)
                    for h in range(H):
                        nc.vector.tensor_scalar(
                            maskw[:, h, :], mask, wm_b[:, h : h + 1], None, AL.mult
                        )

                    # ---- t'.T into [H, N/2] psum halves -> exp ----
                    ed = su1.tile([H, N], BF16, tag="ed")
                    for hf in range(2):
                        ps_t = ps_su.tile([H, N // 2], FP32, tag="ps_t")
                        for q in range(NJC // 2):
                            jc = hf * (NJC // 2) + q
                            nc.tensor.matmul(
                                ps_t[:, q * JC : (q + 1) * JC],
                                lhsT=wstt[:, 0, H : 2 * H],
                                rhs=xt[0][:, jc * JC : (jc + 1) * JC],
                                start=True,
                                stop=False,
                            )
                            nc.tensor.matmul(
                                ps_t[:, q * JC : (q + 1) * JC],
                                lhsT=wstt[:, 1, H : 2 * H],
                                rhs=xt[1][:, jc * JC : (jc + 1) * JC],
                                start=False,
                                stop=True,
                            )
                        nc.scalar.activation(
                            ed[:, hf * (N // 2) : (hf + 1) * (N // 2)],
                            ps_t, AF.Exp, bias=bd_c[:, 0:1],
                        )
                    nc.sync.dma_start(out=edd, in_=ed)
                    for h in range(H):
                        eng = nc.sync if h % 2 == 0 else nc.scalar
                        eng.dma_start(
                            out=e_all[:, h, :],
                            in_=edd[h : h + 1, :].to_broadcast((P, N)),
                        )

                    # ---- s, t'_own for all blocks, batched small ops ----
                    s64 = su1.tile([P, NB, 2, H], FP32, tag="s64")
                    for b in range(NB):
                        ps8 = ps_s8.tile([P, 2 * H], FP32, tag="ps8")
                        nc.tensor.matmul(
                            ps8,
                            lhsT=xt[0][:, b * P : (b + 1) * P],
                            rhs=wstt[:, 0, :],
                            start=True,
                            stop=False,
                        )
                        nc.tensor.matmul(
                            ps8,
                            lhsT=xt[1][:, b * P : (b + 1) * P],
                            rhs=wstt[:, 1, :],
                            start=False,
                            stop=True,
                        )
                        nc.vector.tensor_copy(
                            s64[:, b, :, :].rearrange("p a h -> p (a h)"), ps8
                        )
                    # a = exp(s) for all blocks in one op
                    nc.scalar.activation(a_all, s64[:, :, 0, :], AF.Exp)
                    v = su2.tile([P, NB, H], FP32, tag="v")
                    nc.vector.tensor_tensor(
                        out=v, in0=s64[:, :, 0, :], in1=s64[:, :, 1, :], op=AL.add
                    )
                    u = su2.tile([P, NB, H], FP32, tag="u")
                    for b in range(NB):
                        nc.vector.tensor_tensor(
                            out=u[:, b, :], in0=v[:, b, :], in1=bdr_b, op=AL.add
                        )
                    nc.scalar.activation(u, u, AF.Exp)
                    ue = su2.tile([P, NB, H], FP32, tag="ue")
                    for b in range(NB):
                        nc.vector.tensor_tensor(
                            out=ue[:, b, :], in0=u[:, b, :], in1=esb, op=AL.mult
                        )
                    m1 = su2.tile([P, NB, H], FP32, tag="m1")
                    nc.vector.tensor_scalar(m1, u, 1.0, None, AL.max)
                    m2 = su2.tile([P, NB, H], FP32, tag="m2")
                    nc.vector.tensor_scalar(m2, ue, 1.0, None, AL.max)
                    nc.vector.tensor_tensor(
                        out=del8, in0=m2, in1=m1, op=AL.subtract
                    )
                    for b in range(NB):
                        nc.vector.tensor_scalar(
                            dd8[:, b, 0:NA], del8[:, b, 0:NA], float(N), None, AL.add
                        )
                        nc.vector.tensor_copy(dd8[:, b, NA:H], del8[:, b, NA:H])

                # ================= steady state =================
                with (
                    tc.tile_pool(name="big", bufs=9) as big,
                    tc.tile_pool(name="dcp", bufs=4) as dcp,
                    tc.tile_pool(name="small", bufs=2) as small,
                    tc.tile_pool(name="ost", bufs=2) as ost,
                    tc.tile_pool(name="mps", bufs=1, space="PSUM") as mps,
                ):
                    def make_block(b):
                        st = {"b": b}
                        st["psum"] = mps.tile(
                            [P, N], FP32, tag="psum", name=f"psum{b}"
                        )
                        st["acc8"] = small.tile(
                            [P, H], FP32, tag="acc8", name=f"acc8_{b}"
                        )
                        st["z8"] = small.tile([P, H], FP32, tag="z8", name=f"z8_{b}")
                        st["r8"] = small.tile([P, H], FP32, tag="r8", name=f"r8_{b}")
                        st["kb"] = small.tile([P, 1], FP32, tag="kb", name=f"kb_{b}")
                        st["stage"] = ost.tile(
                            [P, N], FP32, tag="stage", name=f"stage_{b}"
                        )
                        st["ehs"] = {}
                        st["dcs"] = {}
                        return st

                    def produce(st, h):
                        eh = big.tile([P, N], BF16, tag="eh", name=f"eh{st['b']}_{h}")
                        st["ehs"][h] = eh
                        if h < NA:
                            nc.scalar.activation(
                                eh,
                                e_all[:, h, :],
                                AF.Relu,
                                bias=neg1[:, 0:1],
                                scale=a_all[:, st["b"], h : h + 1],
                                accum_out=st["acc8"][:, h : h + 1],
                            )
                        else:
                            nc.vector.scalar_tensor_tensor(
                                eh,
                                e_all[:, h, :],
                                a_all[:, st["b"], h : h + 1],
                                ones,
                                AL.mult,
                                AL.max,
                                accum_out=st["acc8"][:, h : h + 1],
                            )

                    def zrp(st, h):  # heads h, h+1 together
                        nc.vector.tensor_tensor(
                            out=st["z8"][:, h : h + 2],
                            in0=st["acc8"][:, h : h + 2],
                            in1=dd8[:, st["b"], h : h + 2],
                            op=AL.add,
                        )
                        nc.vector.reciprocal(
                            st["r8"][:, h : h + 2], st["z8"][:, h : h + 2]
                        )

                    def dc(st, h):
                        t = dcp.tile([P, P], BF16, tag="dc", name=f"dc{st['b']}_{h}")
                        st["dcs"][h] = t
                        nc.vector.tensor_scalar(
                            t, maskw[:, h, :], st["r8"][:, h : h + 1], None, AL.mult
                        )

                    def merge(st, h, first, last):
                        for jc in range(NJC):
                            nc.tensor.matmul(
                                st["psum"][:, jc * JC : (jc + 1) * JC],
                                lhsT=st["dcs"][h],
                                rhs=st["ehs"][h][:, jc * JC : (jc + 1) * JC],
                                start=first,
                                stop=last,
                            )

                    def kbias(st):
                        b = st["b"]
                        c8a = small.tile([P, NA], FP32, tag="c8a", name=f"c8a_{b}")
                        k1 = small.tile([P, 1], FP32, tag="k1", name=f"k1_{b}")
                        nc.vector.tensor_tensor(
                            out=c8a, in0=st["r8"][:, 0:NA], in1=wm_b[:, 0:NA],
                            op=AL.mult,
                        )
                        nc.vector.tensor_reduce(
                            k1, c8a, axis=mybir.AxisListType.X, op=AL.add
                        )
                        nc.vector.tensor_tensor(
                            out=st["kb"], in0=k1, in1=bm_c, op=AL.add
                        )

                    def drain_lo(st):
                        nc.scalar.activation(
                            st["stage"][:, 0 : N // 2],
                            st["psum"][:, 0 : N // 2],
                            AF.Identity,
                            bias=st["kb"][:, 0:1],
                        )

                    def drain_hi(st):
                        nc.scalar.activation(
                            st["stage"][:, N // 2 : N],
                            st["psum"][:, N // 2 : N],
                            AF.Identity,
                            bias=st["kb"][:, 0:1],
                        )

                    def tail_dve(st):
                        b = st["b"]
                        c8 = small.tile([P, H], FP32, tag="c8", name=f"c8_{b}")
                        t8 = small.tile([P, H], FP32, tag="t8", name=f"t8_{b}")
                        dlt = small.tile([P, 1], FP32, tag="dlt", name=f"dlt_{b}")
                        nc.vector.tensor_tensor(
                            out=c8, in0=st["r8"], in1=wm_b, op=AL.mult
                        )
                        nc.vector.tensor_tensor(
                            out=t8, in0=c8, in1=del8[:, b, :], op=AL.mult
                        )
                        nc.vector.tensor_reduce(
                            dlt, t8, axis=mybir.AxisListType.X, op=AL.add
                        )
                        nc.vector.scalar_tensor_tensor(
                            st["stage"][:, b * P : (b + 1) * P],
                            maskbf,
                            dlt[:, 0:1],
                            st["stage"][:, b * P : (b + 1) * P],
                            AL.mult,
                            AL.add,
                        )

                    def tail_dma(st):
                        b = st["b"]
                        nc.sync.dma_start(
                            out=out_d[b * P : (b + 1) * P, N // 2 : N],
                            in_=st["stage"][:, N // 2 : N],
                        )
                        nc.sync.dma_start(
                            out=out_d[b * P : (b + 1) * P, 0 : N // 2],
                            in_=st["stage"][:, 0 : N // 2],
                        )

                    prev = None
                    for b in range(NB):
                        st = make_block(b)
                        produce(st, 0)       # Act
                        produce(st, NA)      # DVE
                        if prev is not None:
                            drain_lo(prev)   # Act, after produce(b,0)
                        produce(st, 1)       # Act
                        produce(st, NA + 1)  # DVE
                        if prev is not None:
                            drain_hi(prev)   # Act, after produce(b,1)
                        zrp(st, 0)           # DVE
                        dc(st, 0)
                        dc(st, 1)
                        if prev is not None:
                            tail_dve(prev)
                            tail_dma(prev)
                        merge(st, 0, True, False)
                        merge(st, 1, False, False)
                        zrp(st, NA)
                        dc(st, NA)
                        dc(st, NA + 1)
                        merge(st, NA, False, False)
                        merge(st, NA + 1, False, False)
                        produce(st, 2)       # Act
                        produce(st, NA + 2)  # DVE
                        produce(st, 3)       # Act
                        zrp(st, 2)
                        dc(st, 2)
                        dc(st, 3)
                        kbias(st)
                        merge(st, 2, False, False)
                        merge(st, 3, False, False)
                        produce(st, NA + 3)  # DVE
                        zrp(st, NA + 2)
                        dc(st, NA + 2)
                        dc(st, NA + 3)
                        merge(st, NA + 2, False, False)
                        merge(st, NA + 3, False, True)
                        prev = st
                    drain_lo(prev)
                    drain_hi(prev)
                    tail_dve(prev)
                    tail_dma(prev)

    nc.compile()
    return nc


def _get_nc():
    if "nc" not in _CACHE:
        _CACHE["nc"] = _build()
    return _CACHE["nc"]


def _in_maps(inputs):
    import ml_dtypes

    x = np.ascontiguousarray(np.asarray(inputs["x"], dtype=np.float32))
    W_dist = np.asarray(inputs["W_dist"], dtype=np.float32)
    b_dist = np.asarray(inputs["b_dist"], dtype=np.float32).reshape(H, 1)
    W_merge = np.asarray(inputs["W_merge"], dtype=np.float32).reshape(1, H)
    b_merge = np.asarray(inputs["b_merge"], dtype=np.float32).reshape(1, 1)
    selfbias = np.asarray(inputs["selfbias"], dtype=np.float32).reshape(1, H)
    Ws = W_dist[:, :F]
    Wt = W_dist[:, F:]
    wstt = np.empty((2, P, 2 * H), dtype=np.float32)
    for fh in range(2):
        wstt[fh, :, 0:H] = Ws[:, fh * P : (fh + 1) * P].T
        wstt[fh, :, H : 2 * H] = Wt[:, fh * P : (fh + 1) * P].T
    wstt = wstt.astype(ml_dtypes.bfloat16)
    in_maps = []
    for c in range(NCORES):
        xr = np.roll(x, -c * ROWS, axis=0).astype(ml_dtypes.bfloat16)
        in_maps.append(
            {
                "x": np.ascontiguousarray(xr),
                "ws": wstt,
                "bd": b_dist,
                "bdr": np.ascontiguousarray(b_dist.reshape(1, H)),
                "wm": W_merge,
                "bm": b_merge,
                "sb": selfbias,
            }
        )
    return in_maps


def _assemble(results):
    out = np.empty((N, N), dtype=np.float32)
    for c in range(NCORES):
        out[c * ROWS : (c + 1) * ROWS, :] = np.roll(
            results[c]["out"], c * ROWS, axis=1
        )
    return out


def kernel(x, W_dist, b_dist, W_merge, b_merge, selfbias):
    nc = _get_nc()
    in_maps = _in_maps(
        {
            "x": x,
            "W_dist": W_dist,
            "b_dist": b_dist,
            "W_merge": W_merge,
            "b_merge": b_merge,
            "selfbias": selfbias,
        }
    )
    res = run_bass_kernel_spmd(nc, in_maps, core_ids=list(range(NCORES)))
    return _assemble(res.results)


# revision 13
# speedup vs baseline: 1.6264x; 1.0665x over previous
"""Trainium2 Bass kernel for nn_DAAdj_57114475102829 (GAT-style message passing).

Math (N=4096, F=256, H=8):
  s = x @ Ws.T; t' = x @ Wt.T + b_dist
  z[i,j,h] = s[i,h] + t'[j,h] + (i==j)*selfbias[h]
  out = softmax(relu(z), axis=j) @ W_merge + b_merge

Identity: exp(relu(z)) = max(exp(z),1) = 1 + relu(exp(z)-1), exp(z) = a*e
with a = exp(s) (per-partition scale), e = exp(t') (broadcast row).
Per (row-block, head) ONE fused production instruction:
  Act:  G  = Relu(a*E - 1), accum -> Z-N     (heads 0..NA-1)
  DVE:  EH = max(a*E, 1) via STT, accum -> Z (heads NA..7)
Merge: bf16 PE matmuls psum += diag(wm/Z) @ eh; +1 offset of G-heads folds
into the drain bias K = b_merge + sum_{G-heads} c_h.  Drains are two Act
Identity ops (banks 0-3 / 4-7) software-pipelined into the next block's
instruction stream.  Diagonal selfbias enters as per-row corrections
(denominator Delta pre-add; output delta applied to the staged tile).

Sharding: rows across 8 cores; x rotated per core so the diagonal falls
in columns b*128..(b+1)*128 (bank 0) of each row-block b.
"""
import sys

sys.path.insert(0, "/opt/trn_rl_repo")

import numpy as np
import concourse.bacc as bacc
from concourse import mybir
from concourse.tile import TileContext
from concourse.bass_utils import run_bass_kernel_spmd

N, F, H = 4096, 256, 8
NCORES = 8
ROWS = N // NCORES
P = 128
NB = ROWS // P
JC = 512
NJC = N // JC
FP32 = mybir.dt.float32
BF16 = mybir.dt.bfloat16
AL = mybir.AluOpType
AF = mybir.ActivationFunctionType

NA = 4  # heads produced on Act engine (0..NA-1); DVE gets NA..7

_CACHE = {}


def _build():
    nc = bacc.Bacc("TRN2", target_bir_lowering=False, debug=False, num_devices=NCORES)

    xt_d = nc.dram_tensor("xt", [2, P, N], BF16, kind="ExternalInput")
    ws_d = nc.dram_tensor("ws", [2, P, 2 * H], BF16, kind="ExternalInput")
    bd_d = nc.dram_tensor("bd", [H, 1], FP32, kind="ExternalInput")
    bdr_d = nc.dram_tensor("bdr", [1, H], FP32, kind="ExternalInput")
    wm_d = nc.dram_tensor("wm", [1, H], FP32, kind="ExternalInput")
    bm_d = nc.dram_tensor("bm", [1, 1], FP32, kind="ExternalInput")
    sb_d = nc.dram_tensor("sb", [1, H], FP32, kind="ExternalInput")
    out_d = nc.dram_tensor("out", [ROWS, N], FP32, kind="ExternalOutput")

    with TileContext(nc) as tc:
        with tc.tile_pool(name="persist", bufs=1) as persist:
            e_all = persist.tile([P, H, N], BF16, tag="e_all")
            ones = persist.tile([P, N], BF16, tag="ones")
            mask = persist.tile([P, P], FP32, tag="mask")
            maskbf = persist.tile([P, P], BF16, tag="maskbf")
            maskw = persist.tile([P, H, P], BF16, tag="maskw")
            a_all = persist.tile([P, NB, H], FP32, tag="a_all")
            dd8 = persist.tile([P, NB, H], FP32, tag="dd8")
            del8 = persist.tile([P, NB, H], FP32, tag="del8")
            wm_b = persist.tile([P, H], FP32, tag="wm_b")
            bm_c = persist.tile([P, 1], FP32, tag="bm_c")
            sb_b = persist.tile([P, H], FP32, tag="sb_b")
            bdr_b = persist.tile([P, H], FP32, tag="bdr_b")
            bd_c = persist.tile([H, 1], FP32, tag="bd_c")
            neg1 = persist.tile([P, 1], FP32, tag="neg1")
            esb = persist.tile([P, H], FP32, tag="esb")
            it_p = persist.tile([P, 1], FP32, tag="it_p")
            it_f = persist.tile([P, P], FP32, tag="it_f")
            wstt = persist.tile([P, 2, 2 * H], BF16, tag="wstt", name="wstt")

            with tc.tile_pool(name="dram", bufs=1, space="DRAM") as dpool:
                edd = dpool.tile([H, N], BF16)

                with (
                    tc.tile_pool(name="su1", bufs=1) as su1,
                    tc.tile_pool(name="su2", bufs=2) as su2,
                    tc.tile_pool(name="ps_su", bufs=1, space="PSUM") as ps_su,
                    tc.tile_pool(name="ps_s8", bufs=2, space="PSUM") as ps_s8,
                ):
                    # ---- x.T prepped on host: plain loads, one per ring ----
                    xt = [
                        su1.tile([P, N], BF16, tag=f"xt{fh}", name=f"xt{fh}")
                        for fh in range(2)
                    ]
                    nc.sync.dma_start(out=xt[0], in_=xt_d[0, :, :])
                    nc.scalar.dma_start(out=xt[1], in_=xt_d[1, :, :])

                    # ---- params (scalar-engine ring; sync ring busy) ----
                    nc.scalar.dma_start(
                        out=wstt, in_=ws_d[:, :, :].rearrange("a p c -> p a c")
                    )
                    nc.scalar.dma_start(
                        out=wm_b, in_=wm_d[0:1, :].to_broadcast((P, H))
                    )
                    nc.scalar.dma_start(
                        out=bm_c, in_=bm_d[0:1, :].to_broadcast((P, 1))
                    )
                    nc.scalar.dma_start(
                        out=sb_b, in_=sb_d[0:1, :].to_broadcast((P, H))
                    )
                    nc.scalar.dma_start(
                        out=bdr_b, in_=bdr_d[0:1, :].to_broadcast((P, H))
                    )
                    nc.scalar.dma_start(out=bd_c, in_=bd_d[:, :])

                    nc.vector.memset(neg1, -1.0)
                    nc.vector.memset(ones, 1.0)
                    nc.gpsimd.iota(
                        it_p, [[0, 1]], channel_multiplier=1,
                        allow_small_or_imprecise_dtypes=True,
                    )
                    nc.gpsimd.iota(
                        it_f, [[1, P]], channel_multiplier=0,
                        allow_small_or_imprecise_dtypes=True,
                    )
                    nc.vector.tensor_scalar(
                        mask, it_f, it_p[:, 0:1], None, AL.is_equal
                    )
                    nc.vector.tensor_copy(maskbf, mask)
                    nc.scalar.activation(esb, sb_b, AF.Exp)
                    for h in range(H):
                        nc.vector.tensor_scalar(
                            maskw[:, h, :], mask, wm_b[:, h : h + 1], None, AL.mult
                        )

                    # ---- t'.T into [H, N/2] psum halves -> exp ----
                    ed = su1.tile([H, N], BF16, tag="ed")
                    for hf in range(2):
                        ps_t = ps_su.tile([H, N // 2], FP32, tag="ps_t")
                        for q in range(NJC // 2):
                            jc = hf * (NJC // 2) + q
                            nc.tensor.matmul(
                                ps_t[:, q * JC : (q + 1) * JC],
                                lhsT=wstt[:, 0, H : 2 * H],
                                rhs=xt[0][:, jc * JC : (jc + 1) * JC],
                                start=True,
                                stop=False,
                            )
                            nc.tensor.matmul(
                                ps_t[:, q * JC : (q + 1) * JC],
                                lhsT=wstt[:, 1, H : 2 * H],
                                rhs=xt[1][:, jc * JC : (jc + 1) * JC],
                                start=False,
                                stop=True,
                            )
                        nc.scalar.activation(
                            ed[:, hf * (N // 2) : (hf + 1) * (N // 2)],
                            ps_t, AF.Exp, bias=bd_c[:, 0:1],
                        )
                    nc.sync.dma_start(out=edd, in_=ed)
                    for h in range(H):
                        eng = nc.sync if h % 2 == 0 else nc.scalar
                        eng.dma_start(
                            out=e_all[:, h, :],
                            in_=edd[h : h + 1, :].to_broadcast((P, N)),
                        )

                    # ---- s, t'_own for all blocks, batched small ops ----
                    s64 = su1.tile([P, NB, 2, H], FP32, tag="s64")
                    for b in range(NB):
                        ps8 = ps_s8.tile([P, 2 * H], FP32, tag="ps8")
                        nc.tensor.matmul(
                            ps8,
                            lhsT=xt[0][:, b * P : (b + 1) * P],
                            rhs=wstt[:, 0, :],
                            start=True,
                            stop=False,
                        )
                        nc.tensor.matmul(
                            ps8,
                            lhsT=xt[1][:, b * P : (b + 1) * P],
                            rhs=wstt[:, 1, :],
                            start=False,
                            stop=True,
                        )
                        nc.vector.tensor_copy(
                            s64[:, b, :, :].rearrange("p a h -> p (a h)"), ps8
                        )
                    # a = exp(s) for all blocks in one op
                    nc.scalar.activation(a_all, s64[:, :, 0, :], AF.Exp)
                    v = su2.tile([P, NB, H], FP32, tag="v")
                    nc.vector.tensor_tensor(
                        out=v, in0=s64[:, :, 0, :], in1=s64[:, :, 1, :], op=AL.add
                    )
                    u = su2.tile([P, NB, H], FP32, tag="u")
                    for b in range(NB):
                        nc.vector.tensor_tensor(
                            out=u[:, b, :], in0=v[:, b, :], in1=bdr_b, op=AL.add
                        )
                    nc.scalar.activation(u, u, AF.Exp)
                    ue = su2.tile([P, NB, H], FP32, tag="ue")
                    for b in range(NB):
                        nc.vector.tensor_tensor(
                            out=ue[:, b, :], in0=u[:, b, :], in1=esb, op=AL.mult
                        )
                    m1 = su2.tile([P, NB, H], FP32, tag="m1")
                    nc.vector.tensor_scalar(m1, u, 1.0, None, AL.max)
                    m2 = su2.tile([P, NB, H], FP32, tag="m2")
                    nc.vector.tensor_scalar(m2, ue, 1.0, None, AL.max)
                    nc.vector.tensor_tensor(
                        out=del8, in0=m2, in1=m1, op=AL.subtract
                    )
                    for b in range(NB):
                        nc.vector.tensor_scalar(
                            dd8[:, b, 0:NA], del8[:, b, 0:NA], float(N), None, AL.add
                        )
                        nc.vector.tensor_copy(dd8[:, b, NA:H], del8[:, b, NA:H])

                # ================= steady state =================
                with (
                    tc.tile_pool(name="big", bufs=9) as big,
                    tc.tile_pool(name="dcp", bufs=4) as dcp,
                    tc.tile_pool(name="small", bufs=2) as small,
                    tc.tile_pool(name="ost", bufs=2) as ost,
                    tc.tile_pool(name="mps", bufs=1, space="PSUM") as mps,
                ):
                    def make_block(b):
                        st = {"b": b}
                        st["psum"] = mps.tile(
                            [P, N], FP32, tag="psum", name=f"psum{b}"
                        )
                        st["acc8"] = small.tile(
                            [P, H], FP32, tag="acc8", name=f"acc8_{b}"
                        )
                        st["z8"] = small.tile([P, H], FP32, tag="z8", name=f"z8_{b}")
                        st["r8"] = small.tile([P, H], FP32, tag="r8", name=f"r8_{b}")
                        st["kb"] = small.tile([P, 1], FP32, tag="kb", name=f"kb_{b}")
                        st["stage"] = ost.tile(
                            [P, N], FP32, tag="stage", name=f"stage_{b}"
                        )
                        st["ehs"] = {}
                        st["dcs"] = {}
                        return st

                    def produce(st, h):
                        eh = big.tile([P, N], BF16, tag="eh", name=f"eh{st['b']}_{h}")
                        st["ehs"][h] = eh
                        if h < NA:
                            nc.scalar.activation(
                                eh,
                                e_all[:, h, :],
                                AF.Relu,
                                bias=neg1[:, 0:1],
                                scale=a_all[:, st["b"], h : h + 1],
                                accum_out=st["acc8"][:, h : h + 1],
                            )
                        else:
                            nc.vector.scalar_tensor_tensor(
                                eh,
                                e_all[:, h, :],
                                a_all[:, st["b"], h : h + 1],
                                ones,
                                AL.mult,
                                AL.max,
                                accum_out=st["acc8"][:, h : h + 1],
                            )

                    def zrp(st, h):  # heads h, h+1 together
                        nc.vector.tensor_tensor(
                            out=st["z8"][:, h : h + 2],
                            in0=st["acc8"][:, h : h + 2],
                            in1=dd8[:, st["b"], h : h + 2],
                            op=AL.add,
                        )
                        nc.vector.reciprocal(
                            st["r8"][:, h : h + 2], st["z8"][:, h : h + 2]
                        )

                    def dc(st, h):
                        t = dcp.tile([P, P], BF16, tag="dc", name=f"dc{st['b']}_{h}")
                        st["dcs"][h] = t
                        nc.vector.tensor_scalar(
                            t, maskw[:, h, :], st["r8"][:, h : h + 1], None, AL.mult
                        )

                    def merge(st, h, first, last):
                        for jc in range(NJC):
                            nc.tensor.matmul(
                                st["psum"][:, jc * JC : (jc + 1) * JC],
                                lhsT=st["dcs"][h],
                                rhs=st["ehs"][h][:, jc * JC : (jc + 1) * JC],
                                start=first,
                                stop=last,
                            )

                    def kbias(st):
                        b = st["b"]
                        c8a = small.tile([P, NA], FP32, tag="c8a", name=f"c8a_{b}")
                        k1 = small.tile([P, 1], FP32, tag="k1", name=f"k1_{b}")
                        nc.vector.tensor_tensor(
                            out=c8a, in0=st["r8"][:, 0:NA], in1=wm_b[:, 0:NA],
                            op=AL.mult,
                        )
                        nc.vector.tensor_reduce(
                            k1, c8a, axis=mybir.AxisListType.X, op=AL.add
                        )
                        nc.vector.tensor_tensor(
                            out=st["kb"], in0=k1, in1=bm_c, op=AL.add
                        )

                    def drain_lo(st):
                        nc.scalar.activation(
                            st["stage"][:, 0 : N // 2],
                            st["psum"][:, 0 : N // 2],
                            AF.Identity,
                            bias=st["kb"][:, 0:1],
                        )

                    def drain_hi(st):
                        nc.scalar.activation(
                            st["stage"][:, N // 2 : N],
                            st["psum"][:, N // 2 : N],
                            AF.Identity,
                            bias=st["kb"][:, 0:1],
                        )

                    def tail_dve(st):
                        b = st["b"]
                        c8 = small.tile([P, H], FP32, tag="c8", name=f"c8_{b}")
                        t8 = small.tile([P, H], FP32, tag="t8", name=f"t8_{b}")
                        dlt = small.tile([P, 1], FP32, tag="dlt", name=f"dlt_{b}")
                        nc.vector.tensor_tensor(
                            out=c8, in0=st["r8"], in1=wm_b, op=AL.mult
                        )
                        nc.vector.tensor_tensor(
                            out=t8, in0=c8, in1=del8[:, b, :], op=AL.mult
                        )
                        nc.vector.tensor_reduce(
                            dlt, t8, axis=mybir.AxisListType.X, op=AL.add
                        )
                        nc.vector.scalar_tensor_tensor(
                            st["stage"][:, b * P : (b + 1) * P],
                            maskbf,
                            dlt[:, 0:1],
                            st["stage"][:, b * P : (b + 1) * P],
                            AL.mult,
                            AL.add,
                        )

                    def tail_dma(st):
                        b = st["b"]
                        nc.sync.dma_start(
                            out=out_d[b * P : (b + 1) * P, N // 2 : N],
                            in_=st["stage"][:, N // 2 : N],
                        )
                        nc.sync.dma_start(
                            out=out_d[b * P : (b + 1) * P, 0 : N // 2],
                            in_=st["stage"][:, 0 : N // 2],
                        )

                    prev = None
                    for b in range(NB):
                        st = make_block(b)
                        produce(st, 0)       # Act
                        produce(st, NA)      # DVE
                        if prev is not None:
                            drain_lo(prev)   # Act, after produce(b,0)
                        produce(st, 1)       # Act
                        produce(st, NA + 1)  # DVE
                        if prev is not None:
                            drain_hi(prev)   # Act, after produce(b,1)
                        zrp(st, 0)           # DVE
                        dc(st, 0)
                        dc(st, 1)
                        if prev is not None:
                            tail_dve(prev)
                            tail_dma(prev)
                        merge(st, 0, True, False)
                        merge(st, 1, False, False)
                        zrp(st, NA)
                        dc(st, NA)
                        dc(st, NA + 1)
                        merge(st, NA, False, False)
                        merge(st, NA + 1, False, False)
                        produce(st, 2)       # Act
                        produce(st, NA + 2)  # DVE
                        produce(st, 3)       # Act
                        zrp(st, 2)
                        dc(st, 2)
                        dc(st, 3)
                        kbias(st)
                        merge(st, 2, False, False)
                        merge(st, 3, False, False)
                        produce(st, NA + 3)  # DVE
                        zrp(st, NA + 2)
                        dc(st, NA + 2)
                        dc(st, NA + 3)
                        merge(st, NA + 2, False, False)
                        merge(st, NA + 3, False, True)
                        prev = st
                    drain_lo(prev)
                    drain_hi(prev)
                    tail_dve(prev)
                    tail_dma(prev)

    nc.compile()
    return nc


def _get_nc():
    if "nc" not in _CACHE:
        _CACHE["nc"] = _build()
    return _CACHE["nc"]


def _in_maps(inputs):
    import ml_dtypes

    x = np.ascontiguousarray(np.asarray(inputs["x"], dtype=np.float32))
    W_dist = np.asarray(inputs["W_dist"], dtype=np.float32)
    b_dist = np.asarray(inputs["b_dist"], dtype=np.float32).reshape(H, 1)
    W_merge = np.asarray(inputs["W_merge"], dtype=np.float32).reshape(1, H)
    b_merge = np.asarray(inputs["b_merge"], dtype=np.float32).reshape(1, 1)
    selfbias = np.asarray(inputs["selfbias"], dtype=np.float32).reshape(1, H)
    Ws = W_dist[:, :F]
    Wt = W_dist[:, F:]
    wstt = np.empty((2, P, 2 * H), dtype=np.float32)
    for fh in range(2):
        wstt[fh, :, 0:H] = Ws[:, fh * P : (fh + 1) * P].T
        wstt[fh, :, H : 2 * H] = Wt[:, fh * P : (fh + 1) * P].T
    wstt = wstt.astype(ml_dtypes.bfloat16)
    in_maps = []
    xb = x.astype(ml_dtypes.bfloat16)
    for c in range(NCORES):
        xr = np.roll(xb, -c * ROWS, axis=0)
        xtc = np.empty((2, P, N), dtype=ml_dtypes.bfloat16)
        xtc[0] = xr[:, 0:P].T
        xtc[1] = xr[:, P : 2 * P].T
        in_maps.append(
            {
                "xt": np.ascontiguousarray(xtc),
                "ws": wstt,
                "bd": b_dist,
                "bdr": np.ascontiguousarray(b_dist.reshape(1, H)),
                "wm": W_merge,
                "bm": b_merge,
                "sb": selfbias,
            }
        )
    return in_maps


def _assemble(results):
    out = np.empty((N, N), dtype=np.float32)
    for c in range(NCORES):
        out[c * ROWS : (c + 1) * ROWS, :] = np.roll(
            results[c]["out"], c * ROWS, axis=1
        )
    return out


def kernel(x, W_dist, b_dist, W_merge, b_merge, selfbias):
    nc = _get_nc()
    in_maps = _in_maps(
        {
            "x": x,
            "W_dist": W_dist,
            "b_dist": b_dist,
            "W_merge": W_merge,
            "b_merge": b_merge,
            "selfbias": selfbias,
        }
    )
    res = run_bass_kernel_spmd(nc, in_maps, core_ids=list(range(NCORES)))
    return _assemble(res.results)


# revision 14
# speedup vs baseline: 1.6381x; 1.0072x over previous
"""Trainium2 Bass kernel for nn_DAAdj_57114475102829 (GAT-style message passing).

Math (N=4096, F=256, H=8):
  s = x @ Ws.T; t' = x @ Wt.T + b_dist
  z[i,j,h] = s[i,h] + t'[j,h] + (i==j)*selfbias[h]
  out = softmax(relu(z), axis=j) @ W_merge + b_merge

Identity: exp(relu(z)) = max(exp(z),1) = 1 + relu(exp(z)-1), exp(z) = a*e
with a = exp(s) (per-partition scale), e = exp(t') (broadcast row).
Per (row-block, head) ONE fused production instruction:
  Act:  G  = Relu(a*E - 1), accum -> Z-N     (heads 0..NA-1)
  DVE:  EH = max(a*E, 1) via STT, accum -> Z (heads NA..7)
Merge: bf16 PE matmuls psum += diag(wm/Z) @ eh; +1 offset of G-heads folds
into the drain bias K = b_merge + sum_{G-heads} c_h.  Drains are two Act
Identity ops (banks 0-3 / 4-7) software-pipelined into the next block's
instruction stream.  Diagonal selfbias enters as per-row corrections
(denominator Delta pre-add; output delta applied to the staged tile).

Sharding: rows across 8 cores; x rotated per core so the diagonal falls
in columns b*128..(b+1)*128 (bank 0) of each row-block b.
"""
import sys

sys.path.insert(0, "/opt/trn_rl_repo")

import numpy as np
import concourse.bacc as bacc
from concourse import mybir
from concourse.tile import TileContext
from concourse.bass_utils import run_bass_kernel_spmd

N, F, H = 4096, 256, 8
NCORES = 8
ROWS = N // NCORES
P = 128
NB = ROWS // P
JC = 512
NJC = N // JC
FP32 = mybir.dt.float32
BF16 = mybir.dt.bfloat16
AL = mybir.AluOpType
AF = mybir.ActivationFunctionType

NA = 4  # heads produced on Act engine (0..NA-1); DVE gets NA..7

_CACHE = {}


def _build():
    nc = bacc.Bacc("TRN2", target_bir_lowering=False, debug=False, num_devices=NCORES)

    xt_d = nc.dram_tensor("xt", [2, P, N], BF16, kind="ExternalInput")
    ws_d = nc.dram_tensor("ws", [2, P, 2 * H], BF16, kind="ExternalInput")
    bd_d = nc.dram_tensor("bd", [H, 1], FP32, kind="ExternalInput")
    bdr_d = nc.dram_tensor("bdr", [1, H], FP32, kind="ExternalInput")
    wm_d = nc.dram_tensor("wm", [1, H], FP32, kind="ExternalInput")
    bm_d = nc.dram_tensor("bm", [1, 1], FP32, kind="ExternalInput")
    sb_d = nc.dram_tensor("sb", [1, H], FP32, kind="ExternalInput")
    out_d = nc.dram_tensor("out", [ROWS, N], FP32, kind="ExternalOutput")

    with TileContext(nc) as tc:
        with tc.tile_pool(name="persist", bufs=1) as persist:
            e_all = persist.tile([P, H, N], BF16, tag="e_all")
            ones = persist.tile([P, N], BF16, tag="ones")
            mask = persist.tile([P, P], FP32, tag="mask")
            maskbf = persist.tile([P, P], BF16, tag="maskbf")
            maskw = persist.tile([P, H, P], BF16, tag="maskw")
            a_all = persist.tile([P, NB, H], FP32, tag="a_all")
            dd8 = persist.tile([P, NB, H], FP32, tag="dd8")
            del8 = persist.tile([P, NB, H], FP32, tag="del8")
            wm_b = persist.tile([P, H], FP32, tag="wm_b")
            bm_c = persist.tile([P, 1], FP32, tag="bm_c")
            sb_b = persist.tile([P, H], FP32, tag="sb_b")
            bdr_b = persist.tile([P, H], FP32, tag="bdr_b")
            bd_c = persist.tile([H, 1], FP32, tag="bd_c")
            neg1 = persist.tile([P, 1], FP32, tag="neg1")
            esb = persist.tile([P, H], FP32, tag="esb")
            it_p = persist.tile([P, 1], FP32, tag="it_p")
            it_f = persist.tile([P, P], FP32, tag="it_f")
            wstt = persist.tile([P, 2, 2 * H], BF16, tag="wstt", name="wstt")

            with tc.tile_pool(name="dram", bufs=1, space="DRAM") as dpool:
                edd = dpool.tile([H, N], BF16)

                with (
                    tc.tile_pool(name="su1", bufs=1) as su1,
                    tc.tile_pool(name="su2", bufs=2) as su2,
                    tc.tile_pool(name="ps_su", bufs=1, space="PSUM") as ps_su,
                    tc.tile_pool(name="ps_s8", bufs=2, space="PSUM") as ps_s8,
                ):
                    # ---- x.T prepped on host: plain loads, one per ring ----
                    xt = [
                        su1.tile([P, N], BF16, tag=f"xt{fh}", name=f"xt{fh}")
                        for fh in range(2)
                    ]
                    nc.sync.dma_start(out=xt[0], in_=xt_d[0, :, :])
                    nc.scalar.dma_start(out=xt[1], in_=xt_d[1, :, :])

                    # ---- params (scalar-engine ring; sync ring busy) ----
                    nc.scalar.dma_start(
                        out=wstt, in_=ws_d[:, :, :].rearrange("a p c -> p a c")
                    )
                    nc.sync.dma_start(
                        out=wm_b, in_=wm_d[0:1, :].to_broadcast((P, H))
                    )
                    nc.sync.dma_start(
                        out=bm_c, in_=bm_d[0:1, :].to_broadcast((P, 1))
                    )
                    nc.sync.dma_start(
                        out=sb_b, in_=sb_d[0:1, :].to_broadcast((P, H))
                    )
                    nc.sync.dma_start(
                        out=bdr_b, in_=bdr_d[0:1, :].to_broadcast((P, H))
                    )
                    nc.sync.dma_start(out=bd_c, in_=bd_d[:, :])

                    nc.vector.memset(neg1, -1.0)
                    nc.vector.memset(ones, 1.0)
                    nc.gpsimd.iota(
                        it_p, [[0, 1]], channel_multiplier=1,
                        allow_small_or_imprecise_dtypes=True,
                    )
                    nc.gpsimd.iota(
                        it_f, [[1, P]], channel_multiplier=0,
                        allow_small_or_imprecise_dtypes=True,
                    )
                    nc.vector.tensor_scalar(
                        mask, it_f, it_p[:, 0:1], None, AL.is_equal
                    )
                    nc.vector.tensor_copy(maskbf, mask)
                    nc.scalar.activation(esb, sb_b, AF.Exp)
                    for h in range(H):
                        nc.vector.tensor_scalar(
                            maskw[:, h, :], mask, wm_b[:, h : h + 1], None, AL.mult
                        )

                    # ---- s, t'_own for all blocks, batched small ops ----
                    s64 = su1.tile([P, NB, 2, H], FP32, tag="s64")
                    for b in range(NB):
                        ps8 = ps_s8.tile([P, 2 * H], FP32, tag="ps8")
                        nc.tensor.matmul(
                            ps8,
                            lhsT=xt[0][:, b * P : (b + 1) * P],
                            rhs=wstt[:, 0, :],
                            start=True,
                            stop=False,
                        )
                        nc.tensor.matmul(
                            ps8,
                            lhsT=xt[1][:, b * P : (b + 1) * P],
                            rhs=wstt[:, 1, :],
                            start=False,
                            stop=True,
                        )
                        nc.vector.tensor_copy(
                            s64[:, b, :, :].rearrange("p a h -> p (a h)"), ps8
                        )
                    # ---- t'.T into [H, N/2] psum halves -> exp ----
                    ed = su1.tile([H, N], BF16, tag="ed")
                    for hf in range(2):
                        ps_t = ps_su.tile([H, N // 2], FP32, tag="ps_t")
                        for q in range(NJC // 2):
                            jc = hf * (NJC // 2) + q
                            nc.tensor.matmul(
                                ps_t[:, q * JC : (q + 1) * JC],
                                lhsT=wstt[:, 0, H : 2 * H],
                                rhs=xt[0][:, jc * JC : (jc + 1) * JC],
                                start=True,
                                stop=False,
                            )
                            nc.tensor.matmul(
                                ps_t[:, q * JC : (q + 1) * JC],
                                lhsT=wstt[:, 1, H : 2 * H],
                                rhs=xt[1][:, jc * JC : (jc + 1) * JC],
                                start=False,
                                stop=True,
                            )
                        nc.scalar.activation(
                            ed[:, hf * (N // 2) : (hf + 1) * (N // 2)],
                            ps_t, AF.Exp, bias=bd_c[:, 0:1],
                        )
                    nc.sync.dma_start(out=edd, in_=ed)
                    for h in [0, 4, 1, 5]:
                        nc.sync.dma_start(
                            out=e_all[:, h, :],
                            in_=edd[h : h + 1, :].to_broadcast((P, N)),
                        )
                    for h in [2, 6, 3, 7]:
                        nc.scalar.dma_start(
                            out=e_all[:, h, :],
                            in_=edd[h : h + 1, :].to_broadcast((P, N)),
                        )

                    # a = exp(s) for all blocks in one op
                    nc.scalar.activation(a_all, s64[:, :, 0, :], AF.Exp)
                    v = su2.tile([P, NB, H], FP32, tag="v")
                    nc.vector.tensor_tensor(
                        out=v, in0=s64[:, :, 0, :], in1=s64[:, :, 1, :], op=AL.add
                    )
                    u = su2.tile([P, NB, H], FP32, tag="u")
                    for b in range(NB):
                        nc.vector.tensor_tensor(
                            out=u[:, b, :], in0=v[:, b, :], in1=bdr_b, op=AL.add
                        )
                    nc.scalar.activation(u, u, AF.Exp)
                    ue = su2.tile([P, NB, H], FP32, tag="ue")
                    for b in range(NB):
                        nc.vector.tensor_tensor(
                            out=ue[:, b, :], in0=u[:, b, :], in1=esb, op=AL.mult
                        )
                    m1 = su2.tile([P, NB, H], FP32, tag="m1")
                    nc.vector.tensor_scalar(m1, u, 1.0, None, AL.max)
                    m2 = su2.tile([P, NB, H], FP32, tag="m2")
                    nc.vector.tensor_scalar(m2, ue, 1.0, None, AL.max)
                    nc.vector.tensor_tensor(
                        out=del8, in0=m2, in1=m1, op=AL.subtract
                    )
                    for b in range(NB):
                        nc.vector.tensor_scalar(
                            dd8[:, b, 0:NA], del8[:, b, 0:NA], float(N), None, AL.add
                        )
                        nc.vector.tensor_copy(dd8[:, b, NA:H], del8[:, b, NA:H])

                # ================= steady state =================
                with (
                    tc.tile_pool(name="big", bufs=12) as big,
                    tc.tile_pool(name="dcp", bufs=12) as dcp,
                    tc.tile_pool(name="small", bufs=2) as small,
                    tc.tile_pool(name="ost", bufs=1) as ost,
                    tc.tile_pool(name="mps", bufs=1, space="PSUM") as mps,
                ):
                    def make_block(b):
                        st = {"b": b}
                        st["psum"] = mps.tile(
                            [P, N], FP32, tag="psum", name=f"psum{b}"
                        )
                        st["acc8"] = small.tile(
                            [P, H], FP32, tag="acc8", name=f"acc8_{b}"
                        )
                        st["z8"] = small.tile([P, H], FP32, tag="z8", name=f"z8_{b}")
                        st["r8"] = small.tile([P, H], FP32, tag="r8", name=f"r8_{b}")
                        st["kb"] = small.tile([P, 1], FP32, tag="kb", name=f"kb_{b}")
                        st["stage"] = ost.tile(
                            [P, N], FP32, tag="stage", name=f"stage_{b}"
                        )
                        st["ehs"] = {}
                        st["dcs"] = {}
                        return st

                    def produce(st, h):
                        eh = big.tile([P, N], BF16, tag="eh", name=f"eh{st['b']}_{h}")
                        st["ehs"][h] = eh
                        if h < NA:
                            nc.scalar.activation(
                                eh,
                                e_all[:, h, :],
                                AF.Relu,
                                bias=neg1[:, 0:1],
                                scale=a_all[:, st["b"], h : h + 1],
                                accum_out=st["acc8"][:, h : h + 1],
                            )
                        else:
                            nc.vector.scalar_tensor_tensor(
                                eh,
                                e_all[:, h, :],
                                a_all[:, st["b"], h : h + 1],
                                ones,
                                AL.mult,
                                AL.max,
                                accum_out=st["acc8"][:, h : h + 1],
                            )

                    def zrp(st, h):  # heads h, h+1 together
                        nc.vector.tensor_tensor(
                            out=st["z8"][:, h : h + 2],
                            in0=st["acc8"][:, h : h + 2],
                            in1=dd8[:, st["b"], h : h + 2],
                            op=AL.add,
                        )
                        nc.vector.reciprocal(
                            st["r8"][:, h : h + 2], st["z8"][:, h : h + 2]
                        )

                    def dc(st, h):
                        t = dcp.tile([P, P], BF16, tag="dc", name=f"dc{st['b']}_{h}")
                        st["dcs"][h] = t
                        nc.vector.tensor_scalar(
                            t, maskw[:, h, :], st["r8"][:, h : h + 1], None, AL.mult
                        )

                    def merge(st, h, first, last):
                        for jc in range(NJC):
                            nc.tensor.matmul(
                                st["psum"][:, jc * JC : (jc + 1) * JC],
                                lhsT=st["dcs"][h],
                                rhs=st["ehs"][h][:, jc * JC : (jc + 1) * JC],
                                start=first,
                                stop=last,
                            )

                    def kbias(st):
                        b = st["b"]
                        c8a = small.tile([P, NA], FP32, tag="c8a", name=f"c8a_{b}")
                        k1 = small.tile([P, 1], FP32, tag="k1", name=f"k1_{b}")
                        nc.vector.tensor_tensor(
                            out=c8a, in0=st["r8"][:, 0:NA], in1=wm_b[:, 0:NA],
                            op=AL.mult,
                        )
                        nc.vector.tensor_reduce(
                            k1, c8a, axis=mybir.AxisListType.X, op=AL.add
                        )
                        nc.vector.tensor_tensor(
                            out=st["kb"], in0=k1, in1=bm_c, op=AL.add
                        )

                    def drain_lo(st):
                        nc.scalar.activation(
                            st["stage"][:, 0 : N // 2],
                            st["psum"][:, 0 : N // 2],
                            AF.Identity,
                            bias=st["kb"][:, 0:1],
                        )

                    def drain_hi(st):
                        nc.scalar.activation(
                            st["stage"][:, N // 2 : N],
                            st["psum"][:, N // 2 : N],
                            AF.Identity,
                            bias=st["kb"][:, 0:1],
                        )

                    def tail_dve(st):
                        b = st["b"]
                        c8 = small.tile([P, H], FP32, tag="c8", name=f"c8_{b}")
                        t8 = small.tile([P, H], FP32, tag="t8", name=f"t8_{b}")
                        dlt = small.tile([P, 1], FP32, tag="dlt", name=f"dlt_{b}")
                        nc.vector.tensor_tensor(
                            out=c8, in0=st["r8"], in1=wm_b, op=AL.mult
                        )
                        nc.vector.tensor_tensor(
                            out=t8, in0=c8, in1=del8[:, b, :], op=AL.mult
                        )
                        nc.vector.tensor_reduce(
                            dlt, t8, axis=mybir.AxisListType.X, op=AL.add
                        )
                        nc.vector.scalar_tensor_tensor(
                            st["stage"][:, b * P : (b + 1) * P],
                            maskbf,
                            dlt[:, 0:1],
                            st["stage"][:, b * P : (b + 1) * P],
                            AL.mult,
                            AL.add,
                        )

                    def tail_dma(st):
                        b = st["b"]
                        nc.sync.dma_start(
                            out=out_d[b * P : (b + 1) * P, N // 2 : N],
                            in_=st["stage"][:, N // 2 : N],
                        )
                        nc.sync.dma_start(
                            out=out_d[b * P : (b + 1) * P, 0 : N // 2],
                            in_=st["stage"][:, 0 : N // 2],
                        )

                    prev = None
                    for b in range(NB):
                        st = make_block(b)
                        produce(st, 0)       # Act
                        produce(st, NA)      # DVE
                        if prev is not None:
                            drain_lo(prev)   # Act, after produce(b,0)
                        produce(st, 1)       # Act
                        produce(st, NA + 1)  # DVE
                        if prev is not None:
                            drain_hi(prev)   # Act, after produce(b,1)
                        zrp(st, 0)           # DVE
                        dc(st, 0)
                        dc(st, 1)
                        if prev is not None:
                            tail_dve(prev)
                            tail_dma(prev)
                        zrp(st, NA)
                        dc(st, NA)
                        dc(st, NA + 1)
                        produce(st, 2)       # Act
                        produce(st, NA + 2)  # DVE
                        produce(st, 3)       # Act
                        zrp(st, 2)
                        dc(st, 2)
                        dc(st, 3)
                        kbias(st)
                        produce(st, NA + 3)  # DVE
                        zrp(st, NA + 2)
                        dc(st, NA + 2)
                        dc(st, NA + 3)
                        # dense merge burst: keeps the PE warm (HAM)
                        horder = [0, 1, NA, NA + 1, 2, 3, NA + 2, NA + 3]
                        for k, h in enumerate(horder):
                            merge(st, h, k == 0, k == len(horder) - 1)
                        prev = st
                    drain_lo(prev)
                    drain_hi(prev)
                    tail_dve(prev)
                    tail_dma(prev)

    nc.compile()
    return nc


def _get_nc():
    if "nc" not in _CACHE:
        _CACHE["nc"] = _build()
    return _CACHE["nc"]


def _in_maps(inputs):
    import ml_dtypes

    x = np.ascontiguousarray(np.asarray(inputs["x"], dtype=np.float32))
    W_dist = np.asarray(inputs["W_dist"], dtype=np.float32)
    b_dist = np.asarray(inputs["b_dist"], dtype=np.float32).reshape(H, 1)
    W_merge = np.asarray(inputs["W_merge"], dtype=np.float32).reshape(1, H)
    b_merge = np.asarray(inputs["b_merge"], dtype=np.float32).reshape(1, 1)
    selfbias = np.asarray(inputs["selfbias"], dtype=np.float32).reshape(1, H)
    Ws = W_dist[:, :F]
    Wt = W_dist[:, F:]
    wstt = np.empty((2, P, 2 * H), dtype=np.float32)
    for fh in range(2):
        wstt[fh, :, 0:H] = Ws[:, fh * P : (fh + 1) * P].T
        wstt[fh, :, H : 2 * H] = Wt[:, fh * P : (fh + 1) * P].T
    wstt = wstt.astype(ml_dtypes.bfloat16)
    in_maps = []
    xb = x.astype(ml_dtypes.bfloat16)
    for c in range(NCORES):
        xr = np.roll(xb, -c * ROWS, axis=0)
        xtc = np.empty((2, P, N), dtype=ml_dtypes.bfloat16)
        xtc[0] = xr[:, 0:P].T
        xtc[1] = xr[:, P : 2 * P].T
        in_maps.append(
            {
                "xt": np.ascontiguousarray(xtc),
                "ws": wstt,
                "bd": b_dist,
                "bdr": np.ascontiguousarray(b_dist.reshape(1, H)),
                "wm": W_merge,
                "bm": b_merge,
                "sb": selfbias,
            }
        )
    return in_maps


def _assemble(results):
    out = np.empty((N, N), dtype=np.float32)
    for c in range(NCORES):
        out[c * ROWS : (c + 1) * ROWS, :] = np.roll(
            results[c]["out"], c * ROWS, axis=1
        )
    return out


def kernel(x, W_dist, b_dist, W_merge, b_merge, selfbias):
    nc = _get_nc()
    in_maps = _in_maps(
        {
            "x": x,
            "W_dist": W_dist,
            "b_dist": b_dist,
            "W_merge": W_merge,
            "b_merge": b_merge,
            "selfbias": selfbias,
        }
    )
    res = run_bass_kernel_spmd(nc, in_maps, core_ids=list(range(NCORES)))
    return _assemble(res.results)


# revision 15
# speedup vs baseline: 1.8217x; 1.1121x over previous
"""Trainium2 Bass kernel for nn_DAAdj_57114475102829 (GAT-style message passing).

Math (N=4096, F=256, H=8):
  s = x @ Ws.T; t' = x @ Wt.T + b_dist
  z[i,j,h] = s[i,h] + t'[j,h] + (i==j)*selfbias[h]
  out = softmax(relu(z), axis=j) @ W_merge + b_merge

Identity: exp(relu(z)) = max(exp(z),1) = 1 + relu(exp(z)-1), exp(z) = a*e
with a = exp(s) (per-partition scale), e = exp(t') (broadcast row).
Per (row-block, head) ONE fused production instruction:
  Act:  G  = Relu(a*E - 1), accum -> Z-N     (heads 0..NA-1)
  DVE:  EH = max(a*E, 1) via STT, accum -> Z (heads NA..7)
Merge: bf16 PE matmuls psum += diag(wm/Z) @ eh; +1 offset of G-heads folds
into the drain bias K = b_merge + sum_{G-heads} c_h.  Drains are two Act
Identity ops (banks 0-3 / 4-7) software-pipelined into the next block's
instruction stream.  Diagonal selfbias enters as per-row corrections
(denominator Delta pre-add; output delta applied to the staged tile).

Sharding: rows across 8 cores; x rotated per core so the diagonal falls
in columns b*128..(b+1)*128 (bank 0) of each row-block b.
"""
import sys

sys.path.insert(0, "/opt/trn_rl_repo")

import numpy as np
import concourse.bacc as bacc
from concourse import mybir
from concourse.tile import TileContext
from concourse.bass_utils import run_bass_kernel_spmd

N, F, H = 4096, 256, 8
NCORES = 8
ROWS = N // NCORES
P = 128
NB = ROWS // P
JC = 512
NJC = N // JC
FP32 = mybir.dt.float32
BF16 = mybir.dt.bfloat16
AL = mybir.AluOpType
AF = mybir.ActivationFunctionType

NA = 4  # heads produced on Act engine (0..NA-1); DVE gets NA..7

_CACHE = {}


def _build():
    nc = bacc.Bacc("TRN2", target_bir_lowering=False, debug=False, num_devices=NCORES)

    xt_d = nc.dram_tensor("xt", [2, P, N], BF16, kind="ExternalInput")
    ws_d = nc.dram_tensor("ws", [2, P, 2 * H], BF16, kind="ExternalInput")
    bd_d = nc.dram_tensor("bd", [H, 1], FP32, kind="ExternalInput")
    bdr_d = nc.dram_tensor("bdr", [1, H], FP32, kind="ExternalInput")
    wm_d = nc.dram_tensor("wm", [1, H], FP32, kind="ExternalInput")
    bm_d = nc.dram_tensor("bm", [1, 1], FP32, kind="ExternalInput")
    sb_d = nc.dram_tensor("sb", [1, H], FP32, kind="ExternalInput")
    out_d = nc.dram_tensor("out", [ROWS, N], FP32, kind="ExternalOutput")

    with TileContext(nc) as tc:
        with tc.tile_pool(name="persist", bufs=1) as persist:
            e_all = persist.tile([P, H, N], BF16, tag="e_all")
            ones = persist.tile([P, N], BF16, tag="ones")
            mask = persist.tile([P, P], FP32, tag="mask")
            maskbf = persist.tile([P, P], BF16, tag="maskbf")
            maskw = persist.tile([P, H, P], BF16, tag="maskw")
            a_all = persist.tile([P, NB, H], FP32, tag="a_all")
            dd8 = persist.tile([P, NB, H], FP32, tag="dd8")
            del8 = persist.tile([P, NB, H], FP32, tag="del8")
            wm_b = persist.tile([P, H], FP32, tag="wm_b")
            bm_c = persist.tile([P, 1], FP32, tag="bm_c")
            sb_b = persist.tile([P, H], FP32, tag="sb_b")
            bdr_b = persist.tile([P, H], FP32, tag="bdr_b")
            bd_c = persist.tile([H, 1], FP32, tag="bd_c")
            neg1 = persist.tile([P, 1], FP32, tag="neg1")
            esb = persist.tile([P, H], FP32, tag="esb")
            it_p = persist.tile([P, 1], FP32, tag="it_p")
            it_f = persist.tile([P, P], FP32, tag="it_f")
            wstt = persist.tile([P, 2, 2 * H], BF16, tag="wstt", name="wstt")

            with tc.tile_pool(name="dram", bufs=1, space="DRAM") as dpool:
                edd = dpool.tile([H, N], BF16)

                with (
                    tc.tile_pool(name="su1", bufs=1) as su1,
                    tc.tile_pool(name="su2", bufs=2) as su2,
                    tc.tile_pool(name="ps_su", bufs=1, space="PSUM") as ps_su,
                    tc.tile_pool(name="ps_s8", bufs=2, space="PSUM") as ps_s8,
                ):
                    # ---- x.T prepped on host: plain loads, one per ring ----
                    xt = [
                        su1.tile([P, N], BF16, tag=f"xt{fh}", name=f"xt{fh}")
                        for fh in range(2)
                    ]
                    nc.sync.dma_start(out=xt[0], in_=xt_d[0, :, :])
                    nc.scalar.dma_start(out=xt[1], in_=xt_d[1, :, :])

                    # ---- params (scalar-engine ring; sync ring busy) ----
                    nc.scalar.dma_start(
                        out=wstt, in_=ws_d[:, :, :].rearrange("a p c -> p a c")
                    )
                    nc.sync.dma_start(
                        out=wm_b, in_=wm_d[0:1, :].to_broadcast((P, H))
                    )
                    nc.sync.dma_start(
                        out=bm_c, in_=bm_d[0:1, :].to_broadcast((P, 1))
                    )
                    nc.sync.dma_start(
                        out=sb_b, in_=sb_d[0:1, :].to_broadcast((P, H))
                    )
                    nc.sync.dma_start(
                        out=bdr_b, in_=bdr_d[0:1, :].to_broadcast((P, H))
                    )
                    nc.sync.dma_start(out=bd_c, in_=bd_d[:, :])

                    nc.vector.memset(neg1, -1.0)
                    nc.vector.memset(ones, 1.0)
                    nc.gpsimd.iota(
                        it_p, [[0, 1]], channel_multiplier=1,
                        allow_small_or_imprecise_dtypes=True,
                    )
                    nc.gpsimd.iota(
                        it_f, [[1, P]], channel_multiplier=0,
                        allow_small_or_imprecise_dtypes=True,
                    )
                    nc.vector.tensor_scalar(
                        mask, it_f, it_p[:, 0:1], None, AL.is_equal
                    )
                    nc.vector.tensor_copy(maskbf, mask)
                    nc.scalar.activation(esb, sb_b, AF.Exp)
                    for h in range(H):
                        nc.vector.tensor_scalar(
                            maskw[:, h, :], mask, wm_b[:, h : h + 1], None, AL.mult
                        )

                    # ---- s, t'_own for all blocks, batched small ops ----
                    s64 = su1.tile([P, NB, 2, H], FP32, tag="s64")
                    for b in range(NB):
                        ps8 = ps_s8.tile([P, 2 * H], FP32, tag="ps8")
                        nc.tensor.matmul(
                            ps8,
                            lhsT=xt[0][:, b * P : (b + 1) * P],
                            rhs=wstt[:, 0, :],
                            start=True,
                            stop=False,
                        )
                        nc.tensor.matmul(
                            ps8,
                            lhsT=xt[1][:, b * P : (b + 1) * P],
                            rhs=wstt[:, 1, :],
                            start=False,
                            stop=True,
                        )
                        nc.vector.tensor_copy(
                            s64[:, b, :, :].rearrange("p a h -> p (a h)"), ps8
                        )
                    # ---- t'.T into [H, N/2] psum halves -> exp ----
                    ed = su1.tile([H, N], BF16, tag="ed")
                    for hf in range(2):
                        ps_t = ps_su.tile([H, N // 2], FP32, tag="ps_t")
                        for q in range(NJC // 2):
                            jc = hf * (NJC // 2) + q
                            nc.tensor.matmul(
                                ps_t[:, q * JC : (q + 1) * JC],
                                lhsT=wstt[:, 0, H : 2 * H],
                                rhs=xt[0][:, jc * JC : (jc + 1) * JC],
                                start=True,
                                stop=False,
                            )
                            nc.tensor.matmul(
                                ps_t[:, q * JC : (q + 1) * JC],
                                lhsT=wstt[:, 1, H : 2 * H],
                                rhs=xt[1][:, jc * JC : (jc + 1) * JC],
                                start=False,
                                stop=True,
                            )
                        nc.scalar.activation(
                            ed[:, hf * (N // 2) : (hf + 1) * (N // 2)],
                            ps_t, AF.Exp, bias=bd_c[:, 0:1],
                        )
                    nc.sync.dma_start(out=edd, in_=ed)
                    for h in [0, 4, 1, 5, 2, 6, 3, 7]:
                        nc.sync.dma_start(
                            out=e_all[:, h, :],
                            in_=edd[h : h + 1, :].to_broadcast((P, N)),
                        )

                    # a = exp(s) for all blocks in one op
                    nc.scalar.activation(a_all, s64[:, :, 0, :], AF.Exp)
                    v = su2.tile([P, NB, H], FP32, tag="v")
                    nc.vector.tensor_tensor(
                        out=v, in0=s64[:, :, 0, :], in1=s64[:, :, 1, :], op=AL.add
                    )
                    u = su2.tile([P, NB, H], FP32, tag="u")
                    for b in range(NB):
                        nc.vector.tensor_tensor(
                            out=u[:, b, :], in0=v[:, b, :], in1=bdr_b, op=AL.add
                        )
                    nc.scalar.activation(u, u, AF.Exp)
                    ue = su2.tile([P, NB, H], FP32, tag="ue")
                    for b in range(NB):
                        nc.vector.tensor_tensor(
                            out=ue[:, b, :], in0=u[:, b, :], in1=esb, op=AL.mult
                        )
                    m1 = su2.tile([P, NB, H], FP32, tag="m1")
                    nc.vector.tensor_scalar(m1, u, 1.0, None, AL.max)
                    m2 = su2.tile([P, NB, H], FP32, tag="m2")
                    nc.vector.tensor_scalar(m2, ue, 1.0, None, AL.max)
                    nc.vector.tensor_tensor(
                        out=del8, in0=m2, in1=m1, op=AL.subtract
                    )
                    for b in range(NB):
                        nc.vector.tensor_scalar(
                            dd8[:, b, 0:NA], del8[:, b, 0:NA], float(N), None, AL.add
                        )
                        nc.vector.tensor_copy(dd8[:, b, NA:H], del8[:, b, NA:H])

                # ================= steady state =================
                with (
                    tc.tile_pool(name="big", bufs=13) as big,
                    tc.tile_pool(name="dcp", bufs=12) as dcp,
                    tc.tile_pool(name="small", bufs=2) as small,
                    tc.tile_pool(name="ost", bufs=1) as ost,
                    tc.tile_pool(name="mps", bufs=1, space="PSUM") as mps,
                ):
                    def make_block(b):
                        st = {"b": b}
                        st["psum"] = mps.tile(
                            [P, N], FP32, tag="psum", name=f"psum{b}"
                        )
                        st["acc8"] = small.tile(
                            [P, H], FP32, tag="acc8", name=f"acc8_{b}"
                        )
                        st["z8"] = small.tile([P, H], FP32, tag="z8", name=f"z8_{b}")
                        st["r8"] = small.tile([P, H], FP32, tag="r8", name=f"r8_{b}")
                        st["kb"] = small.tile([P, 1], FP32, tag="kb", name=f"kb_{b}")
                        st["stage"] = ost.tile(
                            [P, N], FP32, tag="stage", name=f"stage_{b}"
                        )
                        st["ehs"] = {}
                        st["dcs"] = {}
                        return st

                    def produce(st, h):
                        eh = big.tile([P, N], BF16, tag="eh", name=f"eh{st['b']}_{h}")
                        st["ehs"][h] = eh
                        if h < NA:
                            nc.scalar.activation(
                                eh,
                                e_all[:, h, :],
                                AF.Relu,
                                bias=neg1[:, 0:1],
                                scale=a_all[:, st["b"], h : h + 1],
                                accum_out=st["acc8"][:, h : h + 1],
                            )
                        else:
                            nc.vector.scalar_tensor_tensor(
                                eh,
                                e_all[:, h, :],
                                a_all[:, st["b"], h : h + 1],
                                ones,
                                AL.mult,
                                AL.max,
                                accum_out=st["acc8"][:, h : h + 1],
                            )

                    def zrp(st, h):  # heads h, h+1 together
                        nc.vector.tensor_tensor(
                            out=st["z8"][:, h : h + 2],
                            in0=st["acc8"][:, h : h + 2],
                            in1=dd8[:, st["b"], h : h + 2],
                            op=AL.add,
                        )
                        nc.vector.reciprocal(
                            st["r8"][:, h : h + 2], st["z8"][:, h : h + 2]
                        )

                    def dc(st, h):
                        t = dcp.tile([P, P], BF16, tag="dc", name=f"dc{st['b']}_{h}")
                        st["dcs"][h] = t
                        nc.vector.tensor_scalar(
                            t, maskw[:, h, :], st["r8"][:, h : h + 1], None, AL.mult
                        )

                    def merge(st, h, first, last):
                        for jc in range(NJC):
                            nc.tensor.matmul(
                                st["psum"][:, jc * JC : (jc + 1) * JC],
                                lhsT=st["dcs"][h],
                                rhs=st["ehs"][h][:, jc * JC : (jc + 1) * JC],
                                start=first,
                                stop=last,
                            )

                    def kbias(st):
                        b = st["b"]
                        c8a = small.tile([P, NA], FP32, tag="c8a", name=f"c8a_{b}")
                        k1 = small.tile([P, 1], FP32, tag="k1", name=f"k1_{b}")
                        nc.vector.tensor_tensor(
                            out=c8a, in0=st["r8"][:, 0:NA], in1=wm_b[:, 0:NA],
                            op=AL.mult,
                        )
                        nc.vector.tensor_reduce(
                            k1, c8a, axis=mybir.AxisListType.X, op=AL.add
                        )
                        nc.vector.tensor_tensor(
                            out=st["kb"], in0=k1, in1=bm_c, op=AL.add
                        )

                    def drain_lo(st):
                        nc.scalar.activation(
                            st["stage"][:, 0 : N // 2],
                            st["psum"][:, 0 : N // 2],
                            AF.Identity,
                            bias=st["kb"][:, 0:1],
                        )

                    def drain_hi(st):
                        nc.scalar.activation(
                            st["stage"][:, N // 2 : N],
                            st["psum"][:, N // 2 : N],
                            AF.Identity,
                            bias=st["kb"][:, 0:1],
                        )

                    def tail_dve(st):
                        b = st["b"]
                        c8 = small.tile([P, H], FP32, tag="c8", name=f"c8_{b}")
                        t8 = small.tile([P, H], FP32, tag="t8", name=f"t8_{b}")
                        dlt = small.tile([P, 1], FP32, tag="dlt", name=f"dlt_{b}")
                        nc.vector.tensor_tensor(
                            out=c8, in0=st["r8"], in1=wm_b, op=AL.mult
                        )
                        nc.vector.tensor_tensor(
                            out=t8, in0=c8, in1=del8[:, b, :], op=AL.mult
                        )
                        nc.vector.tensor_reduce(
                            dlt, t8, axis=mybir.AxisListType.X, op=AL.add
                        )
                        nc.vector.scalar_tensor_tensor(
                            st["stage"][:, b * P : (b + 1) * P],
                            maskbf,
                            dlt[:, 0:1],
                            st["stage"][:, b * P : (b + 1) * P],
                            AL.mult,
                            AL.add,
                        )

                    def tail_dma(st):
                        b = st["b"]
                        nc.sync.dma_start(
                            out=out_d[b * P : (b + 1) * P, N // 2 : N],
                            in_=st["stage"][:, N // 2 : N],
                        )
                        nc.sync.dma_start(
                            out=out_d[b * P : (b + 1) * P, 0 : N // 2],
                            in_=st["stage"][:, 0 : N // 2],
                        )

                    prev = None
                    for b in range(NB):
                        st = make_block(b)
                        produce(st, 0)       # Act
                        produce(st, NA)      # DVE
                        if prev is not None:
                            drain_lo(prev)   # Act, after produce(b,0)
                        produce(st, 1)       # Act
                        produce(st, NA + 1)  # DVE
                        if prev is not None:
                            drain_hi(prev)   # Act, after produce(b,1)
                        zrp(st, 0)           # DVE
                        dc(st, 0)
                        dc(st, 1)
                        if prev is not None:
                            tail_dve(prev)
                            tail_dma(prev)
                        zrp(st, NA)
                        dc(st, NA)
                        dc(st, NA + 1)
                        for k, h in enumerate([0, 1, NA, NA + 1]):
                            merge(st, h, k == 0, False)
                        produce(st, 2)       # Act
                        produce(st, NA + 2)  # DVE
                        produce(st, 3)       # Act
                        zrp(st, 2)
                        dc(st, 2)
                        dc(st, 3)
                        kbias(st)
                        produce(st, NA + 3)  # DVE
                        zrp(st, NA + 2)
                        dc(st, NA + 2)
                        dc(st, NA + 3)
                        for k, h in enumerate([2, 3, NA + 2, NA + 3]):
                            merge(st, h, False, k == 3)
                        prev = st
                    drain_lo(prev)
                    drain_hi(prev)
                    tail_dve(prev)
                    tail_dma(prev)

    nc.compile()
    return nc


def _get_nc():
    if "nc" not in _CACHE:
        _CACHE["nc"] = _build()
    return _CACHE["nc"]


def _in_maps(inputs):
    import ml_dtypes

    x = np.ascontiguousarray(np.asarray(inputs["x"], dtype=np.float32))
    W_dist = np.asarray(inputs["W_dist"], dtype=np.float32)
    b_dist = np.asarray(inputs["b_dist"], dtype=np.float32).reshape(H, 1)
    W_merge = np.asarray(inputs["W_merge"], dtype=np.float32).reshape(1, H)
    b_merge = np.asarray(inputs["b_merge"], dtype=np.float32).reshape(1, 1)
    selfbias = np.asarray(inputs["selfbias"], dtype=np.float32).reshape(1, H)
    Ws = W_dist[:, :F]
    Wt = W_dist[:, F:]
    wstt = np.empty((2, P, 2 * H), dtype=np.float32)
    for fh in range(2):
        wstt[fh, :, 0:H] = Ws[:, fh * P : (fh + 1) * P].T
        wstt[fh, :, H : 2 * H] = Wt[:, fh * P : (fh + 1) * P].T
    wstt = wstt.astype(ml_dtypes.bfloat16)
    in_maps = []
    xb = x.astype(ml_dtypes.bfloat16)
    for c in range(NCORES):
        xr = np.roll(xb, -c * ROWS, axis=0)
        xtc = np.empty((2, P, N), dtype=ml_dtypes.bfloat16)
        xtc[0] = xr[:, 0:P].T
        xtc[1] = xr[:, P : 2 * P].T
        in_maps.append(
            {
                "xt": np.ascontiguousarray(xtc),
                "ws": wstt,
                "bd": b_dist,
                "bdr": np.ascontiguousarray(b_dist.reshape(1, H)),
                "wm": W_merge,
                "bm": b_merge,
                "sb": selfbias,
            }
        )
    return in_maps


def _assemble(results):
    out = np.empty((N, N), dtype=np.float32)
    for c in range(NCORES):
        out[c * ROWS : (c + 1) * ROWS, :] = np.roll(
            results[c]["out"], c * ROWS, axis=1
        )
    return out


def kernel(x, W_dist, b_dist, W_merge, b_merge, selfbias):
    nc = _get_nc()
    in_maps = _in_maps(
        {
            "x": x,
            "W_dist": W_dist,
            "b_dist": b_dist,
            "W_merge": W_merge,
            "b_merge": b_merge,
            "selfbias": selfbias,
        }
    )
    res = run_bass_kernel_spmd(nc, in_maps, core_ids=list(range(NCORES)))
    return _assemble(res.results)
